# revision 22
# baseline (speedup 1.0000x reference)
"""Trainium2 kernel for nn_GCNRegression: linear-GCN scalar collapse, bf16 edge pipeline.

The model is linear (no activation), so 4 GCN layers + mean-pool +
linear head collapse exactly to scalar propagation through the graph:
    c0 = W1 @ W2 @ W3 @ W4 @ Wl;  s0 = x @ c0
    s_k = dinv * (Adj @ (dinv * s_{k-1})) + b_k . c_k
    out[g] = sum_{v in g} s4[v] / n_max + bl
Runs on 8 NeuronCores. Per round: AllGather bf16 state, gpsimd
local_scatter routing (bf16 streams), PE transposes, PE segment
reduction accumulating fp32 in PSUM. All index arrays are
host-precomputed from the edge list.
"""

import sys

sys.path.insert(0, "/opt/trn_rl_repo")

import numpy as np
import ml_dtypes

BF16 = ml_dtypes.bfloat16

P = 128          # partitions
SEGS = 16        # shard rows (psum partitions)
R4 = P // SEGS   # 8 rows per segment
NW = 2           # windows (= LS2/LS3 call count)
BPW = 10         # main blocks per window
BLKW = BPW       # no ovf block (CAP chosen so nothing spills)
B_TOT = NW * BLKW  # total X2/XT blocks
CAP = R4 * BPW   # capacity per (p_s, w, s_v) cell
NRANGE = 8
SBUDGET = 2046   # bf16 values per S window (local_scatter num_elems limit)
CLS_MAX = 2046


def cdiv(a, b):
    return (a + b - 1) // b


def _cumcount(keys):
    """Rank of each element within its key group (stable, array order)."""
    order = np.argsort(keys, kind="stable")
    sk = keys[order]
    grp_start = np.r_[0, np.flatnonzero(sk[1:] != sk[:-1]) + 1]
    sizes = np.diff(np.r_[grp_start, len(keys)])
    cum = np.arange(len(keys)) - np.repeat(grp_start, sizes)
    out = np.empty(len(keys), np.int64)
    out[order] = cum
    return out


def build_layout(n_nodes, nc):
    csh = cdiv(n_nodes, nc * SEGS)
    sh = SEGS * csh
    npad = nc * sh
    cf = npad // P
    return csh, sh, npad, cf


def relabel(edge_col_deg_src, n_nodes, nc):
    """Shard by original id; within shard sort by in-degree desc; lay
    column-major into [SEGS, CSH]. Returns flat[] over padded ids."""
    deg = edge_col_deg_src
    csh, sh, npad, cf = build_layout(n_nodes, nc)
    flat = np.empty(npad, np.int64)
    for c in range(nc):
        ids = np.arange(c * sh, (c + 1) * sh)
        order = np.argsort(-deg[ids], kind="stable")
        t = np.empty(len(ids), np.int64)
        t[order] = np.arange(len(ids))
        s, cc = t % SEGS, t // SEGS
        flat[ids] = c * sh + s * csh + cc
    return flat, (csh, sh, npad, cf)


def build_core(core, re, ve, layout):
    """Per-core assignment. re/ve: device-flat src/dst positions."""
    csh, sh, npad, cf = layout
    E = len(re)
    p_s = re // cf
    fin = ve - core * sh
    s_v = fin // csh
    c_v = fin % csh

    # ---- window per source ----
    usrc, src_inv, src_cnt = np.unique(re, return_inverse=True, return_counts=True)
    usrc_p = usrc // cf
    so = np.lexsort((-src_cnt, usrc_p))
    rank_in_p = _cumcount(usrc_p[so])
    win_of_usrc = np.empty(len(usrc), np.int64)
    win_of_usrc[so] = rank_in_p % NW
    w_e = win_of_usrc[src_inv]

    # ---- overflow: cap (p_s, w, s_v) cells at CAP ----
    cell = (p_s * NW + w_e) * SEGS + s_v
    crank = _cumcount(cell)
    is_ovf = crank >= CAP

    main = ~is_ovf
    # ---- j for main edges ----
    j_e = np.full(E, -1, np.int64)
    mi = np.flatnonzero(main)
    cnt_vw = _cumcount((ve[mi] * NW + w_e[mi]))
    j_e[mi] = s_v[mi] * R4 + (cnt_vw % R4)

    def psj(idx):
        return (p_s[idx] * NW + w_e[idx]) * P + j_e[idx]

    vwj = {}
    def vwj_key(i, jv):
        return (int(ve[i]) * NW + int(w_e[i])) * P + int(jv)
    for _try in range(300):
        k = psj(mi)
        cnt = np.bincount(k, minlength=P * NW * P)
        rank = _cumcount(k)
        move = np.flatnonzero(rank >= BPW)
        if len(move) == 0:
            break
        if _try == 0:
            vk = (ve[mi] * NW + w_e[mi]) * P + j_e[mi]
            uk, uc = np.unique(vk, return_counts=True)
            vwj = dict(zip(uk.tolist(), uc.tolist()))
        for ii in move:
            i = mi[ii]
            base = s_v[i] * R4
            pw = (p_s[i] * NW + w_e[i]) * P
            best = None
            for r in range(R4):
                jv = base + r
                if jv == j_e[i]:
                    continue
                ld = cnt[pw + jv]
                nv = vwj.get(vwj_key(i, jv), 0)
                key = (nv, ld)
                if ld < BPW and (best is None or key < best[0]):
                    best = (key, jv)
            if best is None:
                loads = [cnt[pw + base + r] for r in range(R4)]
                jv = base + int(np.argmin(loads))
            else:
                jv = best[1]
            vwj[vwj_key(i, j_e[i])] = vwj.get(vwj_key(i, j_e[i]), 1) - 1
            cnt[pw + j_e[i]] -= 1
            j_e[i] = jv
            cnt[pw + jv] += 1
            vwj[vwj_key(i, jv)] = vwj.get(vwj_key(i, jv), 0) + 1
    else:
        raise RuntimeError("j balance failed")
    k = psj(mi)
    assert np.bincount(k, minlength=P * NW * P).max() <= BPW

    # ---- b for main ----
    b_e = np.full(E, -1, np.int64)
    b_e[mi] = w_e[mi] * BLKW + _cumcount(k)

    # ---- main layers: rank within (v, w, j) ----
    l_e = np.full(E, -1, np.int64)
    l_e[mi] = _cumcount((ve[mi] * NW + w_e[mi]) * P + j_e[mi])

    return dict(
        E=E, ve=ve, p_s=p_s, s_v=s_v, c_v=c_v, w_e=w_e, j_e=j_e, b_e=b_e,
        l_e=l_e, main=main, is_ovf_cap=is_ovf,
        usrc=usrc, usrc_p=usrc_p, usrc_q=usrc % cf, d_src=src_cnt,
        win_of_usrc=win_of_usrc, src_inv=src_inv,
    )


def assign_ovf(c, Lcap, rw):
    """Phase B: final overflow set = capacity spills + layer spills; assign
    dense ovf slots (t_o unique per (p_s,w); (t_o,jo,g) collision-free with
    small g). Mutates c."""
    E = c["E"]
    r_v = c["c_v"] // rw
    spill_l = np.zeros(E, bool)
    mi = np.flatnonzero(c["main"])
    spill_l[mi] = c["l_e"][mi] >= Lcap[c["w_e"][mi], r_v[mi]]
    is_ovf = c["is_ovf_cap"] | spill_l
    c["main"] = ~is_ovf
    c["oi"] = oi = np.flatnonzero(is_ovf)
    ve, s_v, p_s, w_e = c["ve"], c["s_v"], c["p_s"], c["w_e"]

    jo = np.zeros(len(oi), np.int64)
    t_o = np.zeros(len(oi), np.int64)
    g_o = np.zeros(len(oi), np.int64)
    # greedy dense assignment, per window
    GMAX = 8
    for w in range(NW):
        sel = np.flatnonzero(w_e[oi] == w)
        used_t = np.zeros((P, P), bool)        # (p_s, t) used
        used_cell = np.zeros((GMAX, P, P), bool)   # (g, t, j) used
        for ii in sel:
            i = oi[ii]
            ps = int(p_s[i]); sv = int(s_v[i])
            placed = False
            for g in range(GMAX):
                for r in range(R4):
                    j = sv * R4 + r
                    # scan t options not used for this (p_s)
                    for t in range(P):
                        tt = (t * 37 + ps * 13 + j * 29) % P
                        if used_t[ps, tt] or used_cell[g, tt, j]:
                            continue
                        used_t[ps, tt] = True
                        used_cell[g, tt, j] = True
                        jo[ii], t_o[ii], g_o[ii] = j, tt, g
                        placed = True
                        break
                    if placed:
                        break
                if placed:
                    break
            assert placed, "ovf assignment failed"
    c["jo"] = jo
    c["t_o"] = t_o
    c["g_o"] = g_o
    c["lo_"] = _cumcount(ve[oi] * P + jo)


def finalize_cores(cores_raw, layout):
    csh, sh, npad, cf = layout
    nc = len(cores_raw)
    dmax = max(int(c["d_src"].max()) for c in cores_raw)

    # class sizes m[w][d]: max over (core, partition)
    m = np.zeros((NW, dmax + 1), np.int64)
    for c in cores_raw:
        cnt = np.zeros((P, NW, dmax + 1), np.int64)
        np.add.at(cnt, (c["usrc_p"], c["win_of_usrc"], c["d_src"]), 1)
        m = np.maximum(m, cnt.max(axis=0))
    m[:, 0] = 0

    x0_off = np.zeros((NW, dmax + 1), np.int64)
    x_off = np.zeros((NW, dmax + 1), np.int64)
    x0_woff = np.zeros(NW + 1, np.int64)
    x_woff = np.zeros(NW + 1, np.int64)
    o0 = o = 0
    expand_list = []
    for w in range(NW):
        x0_woff[w] = o0
        x_woff[w] = o
        for d in range(1, dmax + 1):
            if m[w][d] == 0:
                continue
            x0_off[w][d] = o0
            x_off[w][d] = o
            expand_list.append((int(o0), int(m[w][d]), d, int(o), w))
            o0 += int(m[w][d])
            o += int(m[w][d]) * d
        if o0 % 2:      # keep window boundaries even
            o0 += 1
        if o % 2:
            o += 1
    x0_woff[NW] = o0
    x_woff[NW] = o
    CLS, XW = int(o0), int(o)
    assert CLS <= CLS_MAX, f"CLS={CLS}"

    B = B_TOT            # total X2 blocks (BPW main + 1 ovf per window)
    F = B * P

    # main layer counts per (w, col-range); cap so each S window fits
    rw = cdiv(csh, NRANGE)
    widths = [min(rw, csh - r * rw) for r in range(NRANGE)]
    Lmax = np.zeros((NW, NRANGE), np.int64)
    for c in cores_raw:
        mm_ = c["main"]
        r_v = c["c_v"] // rw
        np.maximum.at(Lmax, (c["w_e"][mm_], r_v[mm_]), c["l_e"][mm_] + 1)
    wa = np.array(widths)
    for w in range(NW):
        while int((Lmax[w] * wa).sum()) > SBUDGET:
            r = int(np.argmax(Lmax[w] * 10000 + wa))
            assert Lmax[w][r] > 1, "cannot fit S window"
            Lmax[w][r] -= 1
    for c in cores_raw:
        assign_ovf(c, Lmax, rw)
        assert len(c["oi"]) == 0, f"overflow edges present: {len(c['oi'])}"
    LOmax = np.zeros(NRANGE, np.int64)
    for c in cores_raw:
        r_v = c["c_v"] // rw
        if len(c["oi"]):
            np.maximum.at(LOmax, r_v[c["oi"]], c["lo_"] + 1)
    assert int((LOmax * wa).sum()) <= SBUDGET, f"ovf S window: {int((LOmax*wa).sum())}"

    G_w = np.ones(NW, np.int64)
    for c in cores_raw:
        if len(c["oi"]):
            np.maximum.at(G_w, c["w_e"][c["oi"]], c["g_o"] + 1)
    assert G_w.max() <= 15, f"G_w={G_w}"
    g_base = np.r_[0, np.cumsum(G_w)]
    G = int(g_base[-1])

    # level-major layout: level l of window w holds the contiguous runs of
    # ranges with Lmax > l, so each (w, l, run) is ONE wide matmul.
    s_off = np.zeros((NW, NRANGE, int(Lmax.max() or 1)), np.int64)
    so_off = np.zeros((NRANGE, int(LOmax.max() or 1)), np.int64)
    s_woff = np.zeros(NW + 2, np.int64)
    so = 0
    mm_w = [[] for _ in range(NW)]      # per-window matmul entries
    mm_ovf = []

    def emit_levels(Lvec, off_arr, out_entries):
        nonlocal so
        for l in range(int(Lvec.max()) if len(Lvec) else 0):
            r = 0
            while r < NRANGE:
                if Lvec[r] <= l:
                    r += 1
                    continue
                r0 = r
                run_w = 0
                while r < NRANGE and Lvec[r] > l:
                    off_arr[r][l] = so + run_w
                    run_w += int(widths[r])
                    r += 1
                out_entries.append((int(so), int(run_w), int(r0 * rw)))
                so += run_w

    for w in range(NW):
        s_woff[w] = so
        emit_levels(Lmax[w], s_off[w], mm_w[w])
        if so % 2:
            so += 1
    s_woff[NW] = so
    emit_levels(LOmax, so_off, mm_ovf)
    if so % 2:
        so += 1
    s_woff[NW + 1] = so
    SW = int(so)
    # split mm entries at psum bank boundaries (512 fp32 per bank)
    def split_banks(lst):
        out = []
        for (so_, wd, po) in lst:
            while wd > 0:
                room = 512 - (po % 512)
                take = min(wd, room)
                out.append((so_, take, po))
                so_ += take; po += take; wd -= take
        return out
    mm_w = [split_banks(x) for x in mm_w]
    mm_ovf = split_banks(mm_ovf)
    for w in range(NW + 1):
        assert (s_woff[w + 1] - s_woff[w]) <= 2046, f"S win {w} too wide"

    meta = dict(
        nc=nc, csh=csh, sh=sh, npad=npad, cf=cf, dmax=dmax,
        CLS=CLS, XW=XW, SW=SW, F=F, B=B, G=G, NRANGE=NRANGE, rw=rw,
        x0_off=x0_off, x_off=x_off, x0_woff=x0_woff, x_woff=x_woff,
        expand_list=expand_list, m=m, widths=widths,
        Lmax=Lmax, LOmax=LOmax, s_off=s_off, so_off=so_off, s_woff=s_woff,
        mm_w=mm_w, mm_ovf=mm_ovf, G_w=G_w, g_base=g_base,
    )
    per_core = [emit_core_arrays(c, meta) for c in cores_raw]
    return meta, per_core


def emit_core_arrays(c, meta):
    cf, csh = meta["cf"], meta["csh"]
    CLS, F, G = meta["CLS"], meta["F"], meta["G"]
    x0_off, x_off = meta["x0_off"], meta["x_off"]
    x_woff, s_woff = meta["x_woff"], meta["s_woff"]
    s_off, so_off = meta["s_off"], meta["so_off"]
    rw = meta["rw"]

    def put(arr, prt, pos, tgt):
        arr[prt, pos] = tgt.astype(np.int16)

    # class rank of each source within (p, w, d)
    cls_key = (c["usrc_p"] * NW + c["win_of_usrc"]) * (int(c["d_src"].max()) + 1) + c["d_src"]
    cls_rank = _cumcount(cls_key)

    # ls1
    ls1 = np.full((P, cf), -1, np.int16)
    tgt = x0_off[c["win_of_usrc"], c["d_src"]] + cls_rank
    assert tgt.max() < CLS
    put(ls1, c["usrc_p"], c["usrc_q"], tgt)

    # X position per edge
    r_in_src = _cumcount(c["src_inv"])
    si = c["src_inv"]
    xpos = x_off[c["w_e"], c["d_src"][si]] + cls_rank[si] * c["d_src"][si] + r_in_src

    ls2 = []
    for w in range(NW):
        wlen = int(x_woff[w + 1] - x_woff[w])
        a2 = np.full((P, wlen), -1, np.int16)
        selm = (c["w_e"] == w) & c["main"]
        xl = xpos[selm] - x_woff[w]
        t2 = (c["b_e"][selm] - w * BLKW) * P + c["j_e"][selm]
        put(a2, c["p_s"][selm], xl, t2)
        om = c["w_e"][c["oi"]] == w          # mask over oi order
        xo = xpos[c["oi"]][om] - x_woff[w]
        to = BPW * P + c["t_o"][om]
        put(a2, c["p_s"][c["oi"]][om], xo, to)
        ls2.append(a2)

    # ls3 (main): input XT[:, w*BLKW*128 : +BPW*128], partition j
    ls3 = []
    r_v = c["c_v"] // rw
    for w in range(NW):
        wlen = BPW * P
        slen = int(s_woff[w + 1] - s_woff[w])
        arr = np.full((P, wlen), -1, np.int16)
        selm = (c["w_e"] == w) & c["main"]
        ipos = (c["b_e"][selm] - w * BLKW) * P + c["p_s"][selm]
        t3 = (
            s_off[w, r_v[selm], c["l_e"][selm]]
            + (c["c_v"][selm] - r_v[selm] * rw)
            - s_woff[w]
        )
        assert len(t3) == 0 or (t3.min() >= 0 and t3.max() < slen)
        put(arr, c["j_e"][selm], ipos, t3)
        ls3.append(arr)

    # lsa call w: input XT ovf block (w*BLKW+BPW) [P, 128] -> XO chunk G_w blocks
    oi = c["oi"]
    lsa = []
    for w in range(NW):
        arr = np.full((P, P), -1, np.int16)
        if len(oi):
            sel = c["w_e"][oi] == w
            ipos = c["p_s"][oi][sel]
            ta = c["g_o"][sel] * P + c["jo"][sel]
            put(arr, c["t_o"][sel], ipos, ta)
        lsa.append(arr)

    # lsb: XOT [P, G*128] -> SM ovf window at (j*, so_off + col)
    g_base = meta["g_base"]
    slen_o = int(s_woff[NW + 1] - s_woff[NW])
    lsb = np.full((P, G * P), -1, np.int16)
    if len(oi) and slen_o:
        gg = g_base[c["w_e"][oi]] + c["g_o"]
        ipos = gg * P + c["t_o"]
        tb = (
            so_off[r_v[oi], c["lo_"]]
            + (c["c_v"][oi] - r_v[oi] * rw)
            - s_woff[NW]
        )
        assert tb.min() >= 0 and tb.max() < slen_o
        put(lsb, c["jo"], ipos, tb)

    return dict(ls1=ls1, ls2=ls2, ls3=ls3, lsa=lsa, lsb=lsb)


# ──────────────────────────────────────────────────────────────────────
# numpy emulation (bf16 values, 1 int16 elem per value)
# ──────────────────────────────────────────────────────────────────────

def _emu_ls(data_i16, idx_i16, num_elems):
    Pp, n = idx_i16.shape
    assert data_i16.shape == (Pp, n)
    out = np.zeros((Pp, num_elems), np.int16)
    for p in range(Pp):
        ii = idx_i16[p].astype(np.int64)
        valid = ii >= 0
        assert len(np.unique(ii[valid])) == valid.sum(), "dup idx"
        out[p, ii[valid]] = data_i16[p, valid]
    return out


def emulate_round(w_full, meta, arrs):
    """w_full: [P, cf] float32 (will be cast bf16). Returns psum [SEGS, csh] f32."""
    cf, csh = meta["cf"], meta["csh"]
    CLS, XW, SW, F, B, G = (meta[k] for k in ("CLS", "XW", "SW", "F", "B", "G"))
    x_woff, s_woff = meta["x_woff"], meta["s_woff"]

    d16 = np.ascontiguousarray(w_full.astype(BF16)).view(np.int16)
    x0 = _emu_ls(d16, arrs["ls1"], CLS).view(BF16)

    x = np.zeros((P, XW), BF16)
    for (o0, mm, d, o, w) in meta["expand_list"]:
        x[:, o : o + mm * d] = np.repeat(x0[:, o0 : o0 + mm], d, axis=1)

    x2 = np.zeros((P, F), BF16)
    for w in range(NW):
        lo_, hi = int(x_woff[w]), int(x_woff[w + 1])
        seg = np.ascontiguousarray(x[:, lo_:hi]).view(np.int16)
        o = _emu_ls(seg, arrs["ls2"][w], BLKW * P).view(BF16)
        x2[:, w * BLKW * P : (w + 1) * BLKW * P] = o

    xt = np.zeros((P, F), BF16)
    for b in range(B):
        xt[:, b * P : (b + 1) * P] = x2[:, b * P : (b + 1) * P].T

    sm = np.zeros((P, SW), BF16)
    for w in range(NW):
        sl = int(s_woff[w + 1] - s_woff[w])
        seg = np.ascontiguousarray(xt[:, w * BLKW * P : w * BLKW * P + BPW * P]).view(np.int16)
        o = _emu_ls(seg, arrs["ls3"][w], sl).view(BF16)
        sm[:, int(s_woff[w]) : int(s_woff[w]) + sl] = o

    assert int(s_woff[NW + 1] - s_woff[NW]) == 0, "ovf path removed"

    psum = np.zeros((SEGS, csh), np.float32)
    smf = sm.astype(np.float32)
    for lst in (meta["mm_w"][0], meta["mm_w"][1], meta["mm_ovf"]):
        for (so, wd, po) in lst:
            psum[:, po : po + wd] += smf[:, so : so + wd].reshape(SEGS, R4, wd).sum(axis=1)
    return psum


# ─── preprocessing glue ───


def next_pow2(x):
    p = 1
    while p < x:
        p *= 2
    return p


def preprocess(x, edge_index, batch, nc_count=8, n_graphs=128):
    n_nodes = x.shape[0]
    row = np.asarray(edge_index[0], np.int64)
    col = np.asarray(edge_index[1], np.int64)
    batch = np.asarray(batch, np.int64)

    csh, sh, npad, cf = build_layout(n_nodes, nc_count)
    deg = np.bincount(col, minlength=npad).astype(np.int64)
    flat, layout = relabel(deg, n_nodes, nc_count)
    re, ve = flat[row], flat[col]

    cores_raw = []
    for c in range(nc_count):
        m = (ve // sh) == c
        cores_raw.append(build_core(c, re[m], ve[m], layout))
    meta, per_core = finalize_cores(cores_raw, layout)

    # device-order node arrays
    inv = np.empty(npad, np.int64)          # flat -> original id
    inv[flat] = np.arange(npad)
    deg_dev = deg[inv].astype(np.float64)   # deg at device flat position
    batch_dev = np.full(npad, -1, np.int64)
    batch_dev[flat[:n_nodes]] = batch[:n_nodes]

    dinv_dev = np.where(deg_dev > 0, deg_dev ** -0.5, 0.0).astype(np.float32)

    # ---- pooling structures ----
    g0 = np.zeros(nc_count, np.int64)
    ngl = np.zeros(nc_count, np.int64)
    wg_max = 0
    for c in range(nc_count):
        bd = batch_dev[c * sh:(c + 1) * sh]
        real = bd >= 0
        gmin, gmax = (int(bd[real].min()), int(bd[real].max())) if real.any() else (0, 0)
        g0[c], ngl[c] = gmin, gmax - gmin + 1
        fin = np.arange(sh)
        s = fin // csh
        cnt = np.zeros((SEGS, int(ngl[c])), np.int64)
        np.add.at(cnt, (s[real], bd[real] - gmin), 1)
        wg_max = max(wg_max, int(cnt.max()))
    NGLP = int(ngl.max())
    # pool over [128, CP8] layout: partition p = chunk*16 + s, chunk = c // CP8
    CP8 = cdiv(csh, 8)
    pool_idx = []
    gms = [np.zeros((P, P), np.float32) for _ in range(2)]
    w8_max = 0
    ranks = []
    for c in range(nc_count):
        bd = batch_dev[c * sh:(c + 1) * sh]
        fin = np.arange(sh)
        s, cc = fin // csh, fin % csh
        lg = bd - g0[c]
        pp_ = (cc // CP8) * SEGS + s
        pos = cc % CP8
        rank = np.zeros(sh, np.int64)
        real = bd >= 0
        key = pp_ * 4096 + lg
        rank[real] = _cumcount(key[real])
        ranks.append((pp_, pos, lg, rank, real))
        if real.any():
            w8_max = max(w8_max, int(rank[real].max()) + 1)
    W8 = next_pow2(w8_max)
    PH8 = NGLP * W8
    assert PH8 <= 2046, f"pool window {PH8}"
    assert NGLP <= 32
    for c in range(nc_count):
        pp_, pos, lg, rank, real = ranks[c]
        a = np.full((P, CP8), -1, np.int16)
        tgt = lg[real] * W8 + rank[real]
        a[pp_[real], pos[real]] = tgt.astype(np.int16)
        pool_idx.append(a)
        for li in range(NGLP):
            g = g0[c] + li
            if li < int(ngl[c]) and g < n_graphs:
                fp = c * 32 + li
                gms[fp % 2][fp // 2, g] = 1.0

    pool_meta = dict(NGLP=NGLP, W8=W8, PH8=PH8, CP8=CP8, g0=g0)
    return meta, per_core, pool_meta, pool_idx, gms, flat, dinv_dev, layout


def make_inputs(meta, pool_meta, per_core, pool_idx, gms, flat, dinv_dev,
                x, weights, n_max, n_graphs=128):
    """Build per-core in_maps. weights = dict(W1..Wl, b1..bl)."""
    csh, sh, npad, cf = meta["csh"], meta["sh"], meta["npad"], meta["cf"]
    nc_count = meta["nc"]
    n_nodes = x.shape[0]

    # x in device order, transposed: xT_dev[c] = [128, sh], bf16
    xdev = np.zeros((npad, x.shape[1]), np.float32)
    xdev[flat[:n_nodes]] = x
    dinvf = dinv_dev.reshape(P, cf)
    slo = int(meta["s_woff"][NW + 1] - meta["s_woff"][NW])

    wpack = np.zeros((64, 327), np.float32)
    wpack[:, 0:128] = weights["W1"].T
    wpack[:, 128:192] = weights["W2"].T
    wpack[:, 192:256] = weights["W3"].T
    wpack[:, 256:320] = weights["W4"].T
    wpack[:, 320:321] = weights["Wl"].reshape(64, 1)
    for k in range(1, 5):
        wpack[:, 320 + k:321 + k] = np.asarray(weights[f"b{k}"], np.float32).reshape(64, 1)
    wpack[0, 325] = float(np.asarray(weights["bl"]).ravel()[0])
    wpack[0, 326] = 1.0 / np.float32(n_max)
    bfpack = np.zeros((P, 144), np.float32)
    bfpack[:, 0:128] = np.eye(P)
    bfpack[:, 128:144] = np.repeat(np.eye(SEGS), R4, axis=0)
    gmpack = np.concatenate([gms[0], gms[1]], axis=1)

    in_maps = []
    for c in range(nc_count):
        im = dict(
            xT=np.ascontiguousarray(xdev[c * sh:(c + 1) * sh].T).astype(BF16),
            dinvf=dinvf.astype(np.float32),
            dpack=np.concatenate([
                dinv_dev[c * sh:(c + 1) * sh].reshape(SEGS, csh),
                (dinv_dev[c * sh:(c + 1) * sh] ** 2).reshape(SEGS, csh)],
                axis=1).astype(np.float32),
            ls1=per_core[c]["ls1"],
            ls2p=np.concatenate(per_core[c]["ls2"], axis=1),
            ls3p=np.concatenate(per_core[c]["ls3"], axis=1),
            wpack=wpack,
            bfpack=bfpack.astype(BF16),
            gmpack=gmpack.astype(np.float32),
        )
        if slo > 0:
            im["lsb"] = per_core[c]["lsb"]
            for w in range(NW):
                im[f"lsa_{w}"] = per_core[c]["lsa"][w]
        im["pool8"] = pool_idx[c]
        in_maps.append(im)
    return in_maps


def reference_numpy(x, edge_index, batch, weights, n_graphs=128):
    """Direct numpy reference of the original model."""
    row = np.asarray(edge_index[0]); col = np.asarray(edge_index[1])
    N = x.shape[0]
    deg = np.bincount(col, minlength=N).astype(np.float64)
    dinv = np.where(deg > 0, deg ** -0.5, 0.0)
    norm = dinv[row] * dinv[col]
    h = x.astype(np.float64)
    for k in range(1, 5):
        W = weights[f"W{k}"]
        b = weights[f"b{k}"]
        hw = h @ W
        msg = norm[:, None] * hw[row]
        out = np.zeros((N, hw.shape[1]))
        np.add.at(out, col, msg)
        h = out + b
    sums = np.zeros((n_graphs, h.shape[1]))
    np.add.at(sums, batch, h)
    counts = np.bincount(batch, minlength=n_graphs)
    pooled = sums / counts.max()
    return (pooled @ weights["Wl"] + weights["bl"]).astype(np.float32)


# ─── device kernel ───
from contextlib import ExitStack

import concourse.bass as bass
import concourse.tile as tile
from concourse import bacc, mybir

FP32 = mybir.dt.float32
BF = mybir.dt.bfloat16
I16 = mybir.dt.int16
AT = mybir.ActivationFunctionType
OP = mybir.AluOpType


def build_kernel(meta, pool_meta, n_graphs=128):
    csh, sh, npad, cf = meta["csh"], meta["sh"], meta["npad"], meta["cf"]
    CLS, XW, SW, F, B, G = (meta[k] for k in ("CLS", "XW", "SW", "F", "B", "G"))
    x0_woff, x_woff, s_woff = meta["x0_woff"], meta["x_woff"], meta["s_woff"]
    G_w, g_base = meta["G_w"], meta["g_base"]
    mm_w, mm_ovf = meta["mm_w"], meta["mm_ovf"]
    NGLP, W8 = pool_meta["NGLP"], pool_meta["W8"]
    PH8, CP8 = pool_meta["PH8"], pool_meta["CP8"]
    core_ids = list(range(meta["nc"]))

    nc = bacc.Bacc("TRN2", target_bir_lowering=False, debug=False,
                   num_devices=meta["nc"])

    def din(name, shape, dt=FP32):
        return nc.declare_dram_parameter(name, list(shape), dt, isOutput=False)

    slo_pre = int(s_woff[NW + 1] - s_woff[NW])
    xw_tot = int(x_woff[NW])
    # ---- inputs ----
    xT_in = din("xT", [P, sh], BF)
    dinvf_in = din("dinvf", [P, cf])
    dpack_in = din("dpack", [SEGS, 2 * csh])
    ls1_in = din("ls1", [P, cf], I16)
    ls2p_in = din("ls2p", [P, xw_tot], I16)
    ls3p_in = din("ls3p", [P, NW * BPW * P], I16)
    if slo_pre > 0:
        lsa_in = [din(f"lsa_{w}", [P, P], I16) for w in range(NW)]
        lsb_in = din("lsb", [P, G * P], I16)
    pool8_in = din("pool8", [P, CP8], I16)
    wpack_in = din("wpack", [64, 327])
    bfpack_in = din("bfpack", [P, 144], BF)
    gmpack_in = din("gmpack", [P, 256])
    out_ext = nc.declare_dram_parameter("out", [n_graphs], FP32, isOutput=True)

    # ---- internal DRAM ----
    sh_dram = nc.dram_tensor("sh_dram", [sh], BF)
    full_dram = nc.dram_tensor("full_dram", [npad], BF, addr_space="Shared")
    part_dram = nc.dram_tensor("part_dram", [32], FP32)
    warm_in = nc.dram_tensor("warm_in", [32], FP32)
    warm_out = nc.dram_tensor("warm_out", [256], FP32, addr_space="Shared")
    partall_dram = nc.dram_tensor("partall_dram", [256], FP32, addr_space="Shared")

    slo = int(s_woff[NW + 1] - s_woff[NW])

    with tile.TileContext(nc) as tc:
        with ExitStack() as ctx:
            pool = ctx.enter_context(tc.tile_pool(name="p", bufs=1))
            tp = ctx.enter_context(tc.tile_pool(name="tp", bufs=3, space="PSUM"))
            up = ctx.enter_context(tc.tile_pool(name="up", bufs=1, space="PSUM"))

            # persistent tiles
            state = pool.tile([P, cf], BF)
            wbuf = pool.tile([P, cf], BF)
            tbuf = pool.tile([P, cf], FP32)
            dinvf = pool.tile([P, cf], FP32)
            dpk = pool.tile([SEGS, 2 * csh], FP32)
            dinvs = dpk[:, 0:csh]
            dinv2s = dpk[:, csh:2 * csh]
            bdfs = pool.tile([SEGS, csh], FP32)
            x0 = pool.tile([P, CLS], BF)
            xbuf = pool.tile([P, XW], BF)
            x2 = [pool.tile([P, BLKW * P], BF, name=f"x2_{w}") for w in range(NW)]
            xt = [pool.tile([P, BLKW * P], BF, name=f"xt_{w}") for w in range(NW)]
            sm = pool.tile([P, SW], BF)
            xo = pool.tile([P, G * P], BF)
            xot = pool.tile([P, G * P], BF)
            u_bf = pool.tile([SEGS, csh], BF)
            s4_bf = pool.tile([SEGS, CP8 * 8], BF)
            s4r = pool.tile([P, CP8], BF)
            t4 = pool.tile([SEGS, csh], FP32)
            bfp = pool.tile([P, 144], BF)
            ident = bfp[:, 0:128]
            sel = bfp[:, 128:144]
            ones16 = pool.tile([SEGS, 1], FP32)
            ones128 = pool.tile([1, P], FP32)
            gmp = pool.tile([P, 256], FP32)
            gm = [gmp[:, 0:128], gmp[:, 128:256]]
            ls1 = pool.tile([P, cf], I16)
            ls2p = pool.tile([P, xw_tot], I16)
            ls2 = [ls2p[:, int(x_woff[w]):int(x_woff[w + 1])] for w in range(NW)]
            ls3p = pool.tile([P, NW * BPW * P], I16)
            ls3 = [ls3p[:, w * BPW * P:(w + 1) * BPW * P] for w in range(NW)]
            if slo_pre > 0:
                lsa = [pool.tile([P, P], I16, name=f"lsat{w}") for w in range(NW)]
                lsb = pool.tile([P, G * P], I16)
            plidx8 = pool.tile([P, CP8], I16)
            poolb8 = pool.tile([P, PH8], BF)
            poolf8 = pool.tile([P, PH8], FP32)
            ones128c = pool.tile([P, 1], FP32)
            part_sb = pool.tile([1, 32], FP32)
            partall = pool.tile([P, 2], FP32)
            outrow = pool.tile([1, n_graphs], FP32)
            stage = pool.tile([1, sh], BF)
            wpk = pool.tile([64, 327], FP32)
            wts = {
                "w1t": wpk[:, 0:128],
                "w2t": wpk[:, 128:192],
                "w3t": wpk[:, 192:256],
                "w4t": wpk[:, 256:320],
                "wl": wpk[:, 320:321],
            }
            bs = [wpk[:, 321 + k:322 + k] for k in range(4)]
            blt = wpk[0:1, 325:326]
            invn = wpk[0:1, 326:327]
            cvec = {
                "c3": pool.tile([64, 1], FP32, name="c3t"),
                "c2": pool.tile([64, 1], FP32, name="c2t"),
                "c1": pool.tile([64, 1], FP32, name="c1t"),
                "c0": pool.tile([128, 1], FP32, name="c0t"),
            }
            c0b = pool.tile([128, 1], BF)
            betas = pool.tile([1, 4], FP32)
            betas16 = pool.tile([SEGS, 4], FP32)

            # ---- warmup collective + early weight load ----
            warmsb = pool.tile([1, 32], FP32)
            nc.vector.memset(warmsb[:], 0.0)
            nc.vector.memset(ones16[:], 1.0)
            nc.vector.memset(ones128[:], 1.0)
            nc.vector.memset(ones128c[:], 1.0)
            if CP8 * 8 > csh:
                nc.vector.memset(s4_bf[:, csh:CP8 * 8], 0.0)
            nc.sync.dma_start(warm_in[:].rearrange("(a b) -> a b", a=1),
                              warmsb[:])
            nc.sync.dma_start(wpk[:], wpack_in[:])
            nc.gpsimd.collective_compute(
                "AllGather", OP.bypass, replica_groups=[core_ids],
                ins=[warm_in[:]], outs=[warm_out[:]],
            )

            # ---- c chain + betas ----
            pc = tp.tile([128, 8], FP32, tag="ptr")
            nc.tensor.matmul(pc[0:64, 0:1], wts["w4t"][:], wts["wl"][:],
                             start=True, stop=True)
            nc.vector.tensor_copy(cvec["c3"][:], pc[0:64, 0:1])
            nc.tensor.matmul(pc[0:64, 1:2], wts["w3t"][:], cvec["c3"][:],
                             start=True, stop=True)
            nc.vector.tensor_copy(cvec["c2"][:], pc[0:64, 1:2])
            nc.tensor.matmul(pc[0:64, 2:3], wts["w2t"][:], cvec["c2"][:],
                             start=True, stop=True)
            nc.vector.tensor_copy(cvec["c1"][:], pc[0:64, 2:3])
            nc.tensor.matmul(pc[0:128, 3:4], wts["w1t"][:], cvec["c1"][:],
                             start=True, stop=True)
            nc.vector.tensor_copy(cvec["c0"][:], pc[0:128, 3:4])
            nc.vector.tensor_copy(c0b[:], cvec["c0"][:])
            pb = tp.tile([1, 4], FP32, tag="ptr")
            for k, cn in enumerate(["c1", "c2", "c3"]):
                nc.tensor.matmul(pb[0:1, k:k + 1], bs[k][:], cvec[cn][:],
                                 start=True, stop=True)
            nc.tensor.matmul(pb[0:1, 3:4], bs[3][:], wts["wl"][:],
                             start=True, stop=True)
            nc.vector.tensor_copy(betas[:], pb[:])
            pbb = tp.tile([P, 4], FP32, tag="ptr")
            nc.tensor.matmul(pbb[:], ones128[:], betas[:], start=True, stop=True)
            nc.vector.tensor_copy(betas16[:], pbb[0:SEGS, :])

            # ---- s0 = x @ c0 (bf16, 8 chunks, pipelined) ----
            NCH = 16
            chw = sh // NCH
            assert chw % 2 == 0
            pw2 = chw // 2
            qeng = [nc.sync, nc.scalar, nc.gpsimd]
            # load xT in partition-chunks: 8 full-width rows per DMA = few,
            # large descriptors (the DMA queues are descriptor-bound).
            xfull = pool.tile([P, sh], BF)
            for q in range(16):
                qeng[q % 3].dma_start(xfull[8 * q:8 * (q + 1), :],
                                      xT_in[8 * q:8 * (q + 1), :])
            shv = sh_dram[:].rearrange("(a b) -> a b", a=1)
            for q in range(NCH):
                for pi in range(2):
                    ps0 = tp.tile([P, 512], FP32, tag="ptr", name=f"ps0_{q}_{pi}")
                    nc.tensor.matmul(
                        ps0[0:1, 0:pw2], c0b[:],
                        xfull[:, q * chw + pi * pw2: q * chw + (pi + 1) * pw2],
                        start=True, stop=True)
                    eng = nc.vector if (q + pi) % 2 == 0 else nc.scalar
                    if eng is nc.vector:
                        nc.vector.tensor_copy(
                            stage[:, q * chw + pi * pw2: q * chw + (pi + 1) * pw2],
                            ps0[0:1, 0:pw2])
                    else:
                        nc.scalar.activation(
                            stage[:, q * chw + pi * pw2: q * chw + (pi + 1) * pw2],
                            ps0[0:1, 0:pw2], AT.Copy)
                qeng[q % 3].dma_start(
                    shv[:, q * chw:(q + 1) * chw], stage[:, q * chw:(q + 1) * chw])

            # ---- index-table loads: partition-chunked for big descriptors,
            # issue spread across engine queues ----
            ei = 0
            def ldchunk(dst, srcp, nsp):
                nonlocal ei
                step = P // nsp
                for u in range(nsp):
                    qeng[ei % 3].dma_start(dst[u * step:(u + 1) * step, :],
                                           srcp[u * step:(u + 1) * step, :])
                    ei += 1
            ldchunk(ls2p, ls2p_in, 4)
            ldchunk(ls3p, ls3p_in, 4)
            ldchunk(ls1, ls1_in, 2)
            ldchunk(dinvf, dinvf_in, 4)
            nc.sync.dma_start(dpk[:], dpack_in[:])
            nc.scalar.dma_start(bfp[:], bfpack_in[:])
            ldchunk(gmp, gmpack_in, 2)
            if slo_pre > 0:
                for w in range(NW):
                    nc.sync.dma_start(lsa[w][:], lsa_in[w][:])
                nc.scalar.dma_start(lsb[:], lsb_in[:])
            nc.gpsimd.dma_start(plidx8[:], pool8_in[:])

            # ---- state rounds ----
            def allgather_state(dst):
                nc.gpsimd.collective_compute(
                    "AllGather", OP.bypass, replica_groups=[core_ids],
                    ins=[sh_dram[:]], outs=[full_dram[:]],
                )
                nc.sync.dma_start(
                    dst[:], full_dram[:].rearrange("(p c) -> p c", p=P))

            allgather_state(state)

            # PSUM accumulation is bank-scoped: the first matmul into a bank
            # (start=True) clears the bank's has_written bits; later matmuls
            # (start=False) overwrite-on-first-touch / accumulate-where-set
            # per element. So per round emit exactly one start and one stop
            # per bank, regardless of region interleaving.
            bank_total = {}
            for lst in (mm_w + [mm_ovf]):
                for (so, wd, po) in lst:
                    b = po // 512
                    assert (po + wd - 1) // 512 == b
                    bank_total[b] = bank_total.get(b, 0) + 1

            for rnd in range(4):
                if rnd == 0:
                    # w = bf16(state * dinv) in fp32 (receiver side, rnd 0 only)
                    nc.vector.tensor_copy(tbuf[:], state[:])
                    nc.vector.tensor_tensor(wbuf[:], tbuf[:], dinvf[:], OP.mult)
                # LS1
                nc.gpsimd.local_scatter(
                    x0[:].bitcast(I16), wbuf[:].bitcast(I16), ls1[:],
                    channels=P, num_elems=CLS, num_idxs=cf)
                if rnd < 3:
                    # bdfs for this round's output state (off critical path)
                    nc.vector.tensor_scalar(
                        bdfs[:], dinvs[:], betas16[:, rnd:rnd + 1], None, OP.mult)
                # expand + LS2 per window
                for w in range(NW):
                    ei = 0
                    for (o0, mm_, d, o, we) in meta["expand_list"]:
                        if we != w:
                            continue
                        src = x0[:, o0:o0 + mm_].unsqueeze(2).broadcast_to([P, mm_, d])
                        dst = xbuf[:, o:o + mm_ * d].rearrange("p (m d) -> p m d", d=d)
                        if ei % 2 == 0:
                            nc.vector.tensor_copy(dst, src)
                        else:
                            nc.scalar.activation(dst, src, AT.Copy)
                        ei += 1
                    lo_, hi = int(x_woff[w]), int(x_woff[w + 1])
                    nc.gpsimd.local_scatter(
                        x2[w][:].bitcast(I16),
                        xbuf[:, lo_:hi].bitcast(I16), ls2[w][:],
                        channels=P, num_elems=BLKW * P,
                        num_idxs=(hi - lo_))
                    # transposes of this window's blocks (PE, overlaps next LS2)
                    for b0 in range(0, BLKW, 4):
                        nb = min(4, BLKW - b0)
                        pt = tp.tile([P, 512], BF, tag="ptr", name=f"pt{rnd}_{w}_{b0}")
                        for k in range(nb):
                            b = b0 + k
                            nc.tensor.transpose(pt[:, k * P:(k + 1) * P],
                                                x2[w][:, b * P:(b + 1) * P], ident[:])
                        if (b0 // 4) % 2 == 0:
                            nc.vector.tensor_copy(xt[w][:, b0 * P:(b0 + nb) * P],
                                                  pt[:, 0:nb * P])
                        else:
                            nc.scalar.activation(xt[w][:, b0 * P:(b0 + nb) * P],
                                                 pt[:, 0:nb * P], AT.Copy)
                # LS3 + LSA per window; all sel matmuls accumulate into ONE pu
                pu = up.tile([SEGS, csh], FP32, tag="pu", name=f"pu{rnd}")
                bank_seen = {}
                def emit_mms(lst):
                    for (so, wd, po) in lst:
                        b = po // 512
                        seen = bank_seen.get(b, 0)
                        bank_seen[b] = seen + 1
                        nc.tensor.matmul(
                            pu[:, po:po + wd], sel[:], sm[:, so:so + wd],
                            start=(seen == 0),
                            stop=(seen + 1 == bank_total[b]),
                            skip_group_check=True)
                for w in range(NW):
                    sl = int(s_woff[w + 1] - s_woff[w])
                    nc.gpsimd.local_scatter(
                        sm[:, int(s_woff[w]):int(s_woff[w]) + sl].bitcast(I16),
                        xt[w][:].bitcast(I16),
                        ls3[w][:], channels=P, num_elems=sl,
                        num_idxs=BPW * P)
                    if slo > 0:
                        gw = int(G_w[w])
                        gb = int(g_base[w])
                        nc.gpsimd.local_scatter(
                            xo[:, gb * P:(gb + gw) * P].bitcast(I16),
                            xt[w][:, BPW * P:(BPW + 1) * P].bitcast(I16),
                            lsa[w][:], channels=P, num_elems=gw * P,
                            num_idxs=P)
                        # XO transposes for this window's g-blocks (PE)
                        assert gw <= 4
                        pt = tp.tile([P, 512], BF, tag="ptr", name=f"po{rnd}_{w}")
                        for k in range(gw):
                            g = gb + k
                            nc.tensor.transpose(pt[:, k * P:(k + 1) * P],
                                                xo[:, g * P:(g + 1) * P], ident[:])
                        nc.scalar.activation(xot[:, gb * P:(gb + gw) * P],
                                             pt[:, 0:gw * P], AT.Copy)
                    if w == NW - 1:
                        emit_mms([e for e in mm_w[w] if e[2] < 512]
                                 + [e for e in mm_w[w] if e[2] >= 512])
                    else:
                        emit_mms(mm_w[w])
                # ovf: LSB + ovf matmuls
                if slo > 0:
                    nc.gpsimd.local_scatter(
                        sm[:, int(s_woff[NW]):int(s_woff[NW]) + slo].bitcast(I16),
                        xot[:].bitcast(I16), lsb[:],
                        channels=P, num_elems=slo, num_idxs=G * P)
                    emit_mms(mm_ovf)
                assert bank_seen == bank_total

                # sender-side state math: w_next = bf16(u*dinv^2 + beta*dinv)
                # split per psum bank so bank A's math/send overlaps bank B's
                # matmuls.
                if rnd < 3:
                    shv2 = sh_dram[:].rearrange("(s c) -> s c", s=SEGS)
                    for (a, b2) in ((0, 512), (512, csh)):
                        nc.vector.tensor_tensor(
                            t4[:, a:b2], pu[:, a:b2], dinv2s[:, a:b2], OP.mult)
                        nc.vector.tensor_tensor(
                            u_bf[:, a:b2], t4[:, a:b2], bdfs[:, a:b2], OP.add)
                        nc.sync.dma_start(shv2[:, a:b2], u_bf[:, a:b2])
                    allgather_state(wbuf)
                else:
                    nc.vector.tensor_tensor(t4[:], pu[:], dinvs[:], OP.mult)
                    nc.vector.tensor_scalar(
                        s4_bf[:, 0:csh], t4[:], betas16[:, 3:4], None, OP.add)

            # ---- pooling (on [128, CP8] reshaped shard) ----
            for k in range(8):
                qeng[k % 3].dma_start(s4r[k * SEGS:(k + 1) * SEGS, :],
                                      s4_bf[:, k * CP8:(k + 1) * CP8])
            nc.gpsimd.local_scatter(
                poolb8[:].bitcast(I16), s4r[:].bitcast(I16), plidx8[:],
                channels=P, num_elems=PH8, num_idxs=CP8)
            nc.vector.tensor_copy(poolf8[:], poolb8[:])
            wgp = W8
            a = poolf8[:].rearrange("p (g t) -> p g t", t=W8)
            while wgp > 1:
                hw = wgp // 2
                nc.vector.tensor_tensor(
                    a[:, :, 0:hw], a[:, :, 0:hw], a[:, :, hw:wgp], OP.add)
                wgp = hw
            pp = tp.tile([1, 512], FP32, tag="ptr")
            nc.tensor.matmul(pp[0:1, 0:NGLP], ones128c[:], a[:, :, 0],
                             start=True, stop=True)
            nc.vector.memset(part_sb[:], 0.0)
            nc.vector.tensor_copy(part_sb[:, 0:NGLP], pp[0:1, 0:NGLP])
            nc.sync.dma_start(part_dram[:].rearrange("(a b) -> a b", a=1),
                              part_sb[:])
            nc.gpsimd.collective_compute(
                "AllGather", OP.bypass, replica_groups=[core_ids],
                ins=[part_dram[:]], outs=[partall_dram[:]],
            )
            nc.sync.dma_start(partall[:],
                              partall_dram[:].rearrange("(p c) -> p c", p=P))
            po_ = tp.tile([1, n_graphs], FP32, tag="ptr")
            nc.tensor.matmul(po_[:], partall[:, 0:1], gm[0][:],
                             start=True, stop=False)
            nc.tensor.matmul(po_[:], partall[:, 1:2], gm[1][:],
                             start=False, stop=True)
            nc.vector.tensor_copy(outrow[:], po_[:])
            nc.vector.tensor_scalar(outrow[:], outrow[:], invn[0:1, 0:1],
                                    None, OP.mult)
            nc.vector.tensor_scalar(outrow[:], outrow[:], blt[0:1, 0:1],
                                    None, OP.add)
            nc.sync.dma_start(out_ext[:].rearrange("(a b) -> a b", a=1),
                              outrow[:])
    return nc


# ─── entry point ───

def kernel(x, edge_index, batch, W1, b1, W2, b2, W3, b3, W4, b4, Wl, bl):
    from concourse.bass_utils import run_bass_kernel_spmd

    x = np.asarray(x, np.float32)
    edge_index = np.asarray(edge_index)
    batch = np.asarray(batch)
    weights = dict(W1=np.asarray(W1, np.float32), W2=np.asarray(W2, np.float32),
                   W3=np.asarray(W3, np.float32), W4=np.asarray(W4, np.float32),
                   Wl=np.asarray(Wl, np.float32),
                   b1=np.asarray(b1, np.float32), b2=np.asarray(b2, np.float32),
                   b3=np.asarray(b3, np.float32), b4=np.asarray(b4, np.float32),
                   bl=np.asarray(bl, np.float32))
    n_graphs = 128

    meta, per_core, pool_meta, pool_idx, gms, flat, dinv_dev, layout = \
        preprocess(x, edge_index, batch, 8, n_graphs)
    n_max = int(np.bincount(np.asarray(batch, np.int64),
                            minlength=n_graphs).max())
    in_maps = make_inputs(meta, pool_meta, per_core, pool_idx, gms, flat,
                          dinv_dev, x, weights, n_max, n_graphs)
    nc = build_kernel(meta, pool_meta, n_graphs)
    nc.finalize()
    res = run_bass_kernel_spmd(nc, in_maps, core_ids=list(range(8)),
                               trace=False)
    return res.results[0]["out"].reshape(n_graphs, 1).astype(np.float32)


# revision 23
# speedup vs baseline: 1.0580x; 1.0580x over previous
"""Trainium2 kernel for nn_GCNRegression: linear-GCN scalar collapse, bf16 edge pipeline.

The model is linear (no activation), so 4 GCN layers + mean-pool +
linear head collapse exactly to scalar propagation through the graph:
    c0 = W1 @ W2 @ W3 @ W4 @ Wl;  s0 = x @ c0
    s_k = dinv * (Adj @ (dinv * s_{k-1})) + b_k . c_k
    out[g] = sum_{v in g} s4[v] / n_max + bl
Runs on 8 NeuronCores. Per round: AllGather bf16 state, gpsimd
local_scatter routing (bf16 streams), PE transposes, PE segment
reduction accumulating fp32 in PSUM. All index arrays are
host-precomputed from the edge list.
"""

import sys

sys.path.insert(0, "/opt/trn_rl_repo")

import numpy as np
import ml_dtypes

BF16 = ml_dtypes.bfloat16

P = 128          # partitions
SEGS = 16        # shard rows (psum partitions)
R4 = P // SEGS   # 8 rows per segment
NW = 2           # windows (= LS2/LS3 call count)
BPW = 10         # main blocks per window
BLKW = BPW       # no ovf block (CAP chosen so nothing spills)
B_TOT = NW * BLKW  # total X2/XT blocks
CAP = R4 * BPW   # capacity per (p_s, w, s_v) cell
NRANGE = 8
SBUDGET = 2046   # bf16 values per S window (local_scatter num_elems limit)
CLS_MAX = 2046


def cdiv(a, b):
    return (a + b - 1) // b


def _cumcount(keys):
    """Rank of each element within its key group (stable, array order)."""
    order = np.argsort(keys, kind="stable")
    sk = keys[order]
    grp_start = np.r_[0, np.flatnonzero(sk[1:] != sk[:-1]) + 1]
    sizes = np.diff(np.r_[grp_start, len(keys)])
    cum = np.arange(len(keys)) - np.repeat(grp_start, sizes)
    out = np.empty(len(keys), np.int64)
    out[order] = cum
    return out


def build_layout(n_nodes, nc):
    csh = cdiv(n_nodes, nc * SEGS)
    sh = SEGS * csh
    npad = nc * sh
    cf = npad // P
    return csh, sh, npad, cf


def relabel(edge_col_deg_src, n_nodes, nc):
    """Shard by original id; within shard sort by in-degree desc; lay
    column-major into [SEGS, CSH]. Returns flat[] over padded ids."""
    deg = edge_col_deg_src
    csh, sh, npad, cf = build_layout(n_nodes, nc)
    flat = np.empty(npad, np.int64)
    for c in range(nc):
        ids = np.arange(c * sh, (c + 1) * sh)
        order = np.argsort(-deg[ids], kind="stable")
        t = np.empty(len(ids), np.int64)
        t[order] = np.arange(len(ids))
        s, cc = t % SEGS, t // SEGS
        flat[ids] = c * sh + s * csh + cc
    return flat, (csh, sh, npad, cf)


def build_core(core, re, ve, layout):
    """Per-core assignment. re/ve: device-flat src/dst positions."""
    csh, sh, npad, cf = layout
    E = len(re)
    p_s = re // cf
    fin = ve - core * sh
    s_v = fin // csh
    c_v = fin % csh

    # ---- window per source ----
    usrc, src_inv, src_cnt = np.unique(re, return_inverse=True, return_counts=True)
    usrc_p = usrc // cf
    so = np.lexsort((-src_cnt, usrc_p))
    rank_in_p = _cumcount(usrc_p[so])
    win_of_usrc = np.empty(len(usrc), np.int64)
    win_of_usrc[so] = rank_in_p % NW
    w_e = win_of_usrc[src_inv]

    # ---- overflow: cap (p_s, w, s_v) cells at CAP ----
    cell = (p_s * NW + w_e) * SEGS + s_v
    crank = _cumcount(cell)
    is_ovf = crank >= CAP

    main = ~is_ovf
    # ---- j for main edges ----
    j_e = np.full(E, -1, np.int64)
    mi = np.flatnonzero(main)
    cnt_vw = _cumcount((ve[mi] * NW + w_e[mi]))
    j_e[mi] = s_v[mi] * R4 + (cnt_vw % R4)

    def psj(idx):
        return (p_s[idx] * NW + w_e[idx]) * P + j_e[idx]

    vwj = {}
    def vwj_key(i, jv):
        return (int(ve[i]) * NW + int(w_e[i])) * P + int(jv)
    for _try in range(300):
        k = psj(mi)
        cnt = np.bincount(k, minlength=P * NW * P)
        rank = _cumcount(k)
        move = np.flatnonzero(rank >= BPW)
        if len(move) == 0:
            break
        if _try == 0:
            vk = (ve[mi] * NW + w_e[mi]) * P + j_e[mi]
            uk, uc = np.unique(vk, return_counts=True)
            vwj = dict(zip(uk.tolist(), uc.tolist()))
        for ii in move:
            i = mi[ii]
            base = s_v[i] * R4
            pw = (p_s[i] * NW + w_e[i]) * P
            best = None
            for r in range(R4):
                jv = base + r
                if jv == j_e[i]:
                    continue
                ld = cnt[pw + jv]
                nv = vwj.get(vwj_key(i, jv), 0)
                key = (nv, ld)
                if ld < BPW and (best is None or key < best[0]):
                    best = (key, jv)
            if best is None:
                loads = [cnt[pw + base + r] for r in range(R4)]
                jv = base + int(np.argmin(loads))
            else:
                jv = best[1]
            vwj[vwj_key(i, j_e[i])] = vwj.get(vwj_key(i, j_e[i]), 1) - 1
            cnt[pw + j_e[i]] -= 1
            j_e[i] = jv
            cnt[pw + jv] += 1
            vwj[vwj_key(i, jv)] = vwj.get(vwj_key(i, jv), 0) + 1
    else:
        raise RuntimeError("j balance failed")
    k = psj(mi)
    assert np.bincount(k, minlength=P * NW * P).max() <= BPW

    # ---- b for main ----
    b_e = np.full(E, -1, np.int64)
    b_e[mi] = w_e[mi] * BLKW + _cumcount(k)

    # ---- main layers: rank within (v, w, j) ----
    l_e = np.full(E, -1, np.int64)
    l_e[mi] = _cumcount((ve[mi] * NW + w_e[mi]) * P + j_e[mi])

    return dict(
        E=E, ve=ve, p_s=p_s, s_v=s_v, c_v=c_v, w_e=w_e, j_e=j_e, b_e=b_e,
        l_e=l_e, main=main, is_ovf_cap=is_ovf,
        usrc=usrc, usrc_p=usrc_p, usrc_q=usrc % cf, d_src=src_cnt,
        win_of_usrc=win_of_usrc, src_inv=src_inv,
    )


def assign_ovf(c, Lcap, rw):
    """Phase B: final overflow set = capacity spills + layer spills; assign
    dense ovf slots (t_o unique per (p_s,w); (t_o,jo,g) collision-free with
    small g). Mutates c."""
    E = c["E"]
    r_v = c["c_v"] // rw
    spill_l = np.zeros(E, bool)
    mi = np.flatnonzero(c["main"])
    spill_l[mi] = c["l_e"][mi] >= Lcap[c["w_e"][mi], r_v[mi]]
    is_ovf = c["is_ovf_cap"] | spill_l
    c["main"] = ~is_ovf
    c["oi"] = oi = np.flatnonzero(is_ovf)
    ve, s_v, p_s, w_e = c["ve"], c["s_v"], c["p_s"], c["w_e"]

    jo = np.zeros(len(oi), np.int64)
    t_o = np.zeros(len(oi), np.int64)
    g_o = np.zeros(len(oi), np.int64)
    # greedy dense assignment, per window
    GMAX = 8
    for w in range(NW):
        sel = np.flatnonzero(w_e[oi] == w)
        used_t = np.zeros((P, P), bool)        # (p_s, t) used
        used_cell = np.zeros((GMAX, P, P), bool)   # (g, t, j) used
        for ii in sel:
            i = oi[ii]
            ps = int(p_s[i]); sv = int(s_v[i])
            placed = False
            for g in range(GMAX):
                for r in range(R4):
                    j = sv * R4 + r
                    # scan t options not used for this (p_s)
                    for t in range(P):
                        tt = (t * 37 + ps * 13 + j * 29) % P
                        if used_t[ps, tt] or used_cell[g, tt, j]:
                            continue
                        used_t[ps, tt] = True
                        used_cell[g, tt, j] = True
                        jo[ii], t_o[ii], g_o[ii] = j, tt, g
                        placed = True
                        break
                    if placed:
                        break
                if placed:
                    break
            assert placed, "ovf assignment failed"
    c["jo"] = jo
    c["t_o"] = t_o
    c["g_o"] = g_o
    c["lo_"] = _cumcount(ve[oi] * P + jo)


def finalize_cores(cores_raw, layout):
    csh, sh, npad, cf = layout
    nc = len(cores_raw)
    dmax = max(int(c["d_src"].max()) for c in cores_raw)

    # class sizes m[w][d]: max over (core, partition)
    m = np.zeros((NW, dmax + 1), np.int64)
    for c in cores_raw:
        cnt = np.zeros((P, NW, dmax + 1), np.int64)
        np.add.at(cnt, (c["usrc_p"], c["win_of_usrc"], c["d_src"]), 1)
        m = np.maximum(m, cnt.max(axis=0))
    m[:, 0] = 0

    x0_off = np.zeros((NW, dmax + 1), np.int64)
    x_off = np.zeros((NW, dmax + 1), np.int64)
    x0_woff = np.zeros(NW + 1, np.int64)
    x_woff = np.zeros(NW + 1, np.int64)
    o0 = o = 0
    expand_list = []
    for w in range(NW):
        x0_woff[w] = o0
        x_woff[w] = o
        for d in range(1, dmax + 1):
            if m[w][d] == 0:
                continue
            x0_off[w][d] = o0
            x_off[w][d] = o
            expand_list.append((int(o0), int(m[w][d]), d, int(o), w))
            o0 += int(m[w][d])
            o += int(m[w][d]) * d
        if o0 % 2:      # keep window boundaries even
            o0 += 1
        if o % 2:
            o += 1
    x0_woff[NW] = o0
    x_woff[NW] = o
    CLS, XW = int(o0), int(o)
    assert CLS <= CLS_MAX, f"CLS={CLS}"

    B = B_TOT            # total X2 blocks (BPW main + 1 ovf per window)
    F = B * P

    # main layer counts per (w, col-range); cap so each S window fits
    rw = cdiv(csh, NRANGE)
    widths = [min(rw, csh - r * rw) for r in range(NRANGE)]
    Lmax = np.zeros((NW, NRANGE), np.int64)
    for c in cores_raw:
        mm_ = c["main"]
        r_v = c["c_v"] // rw
        np.maximum.at(Lmax, (c["w_e"][mm_], r_v[mm_]), c["l_e"][mm_] + 1)
    wa = np.array(widths)
    for w in range(NW):
        while int((Lmax[w] * wa).sum()) > SBUDGET:
            r = int(np.argmax(Lmax[w] * 10000 + wa))
            assert Lmax[w][r] > 1, "cannot fit S window"
            Lmax[w][r] -= 1
    for c in cores_raw:
        assign_ovf(c, Lmax, rw)
        assert len(c["oi"]) == 0, f"overflow edges present: {len(c['oi'])}"
    LOmax = np.zeros(NRANGE, np.int64)
    for c in cores_raw:
        r_v = c["c_v"] // rw
        if len(c["oi"]):
            np.maximum.at(LOmax, r_v[c["oi"]], c["lo_"] + 1)
    assert int((LOmax * wa).sum()) <= SBUDGET, f"ovf S window: {int((LOmax*wa).sum())}"

    G_w = np.ones(NW, np.int64)
    for c in cores_raw:
        if len(c["oi"]):
            np.maximum.at(G_w, c["w_e"][c["oi"]], c["g_o"] + 1)
    assert G_w.max() <= 15, f"G_w={G_w}"
    g_base = np.r_[0, np.cumsum(G_w)]
    G = int(g_base[-1])

    # level-major layout: level l of window w holds the contiguous runs of
    # ranges with Lmax > l, so each (w, l, run) is ONE wide matmul.
    s_off = np.zeros((NW, NRANGE, int(Lmax.max() or 1)), np.int64)
    so_off = np.zeros((NRANGE, int(LOmax.max() or 1)), np.int64)
    s_woff = np.zeros(NW + 2, np.int64)
    so = 0
    mm_w = [[] for _ in range(NW)]      # per-window matmul entries
    mm_ovf = []

    def emit_levels(Lvec, off_arr, out_entries):
        nonlocal so
        for l in range(int(Lvec.max()) if len(Lvec) else 0):
            r = 0
            while r < NRANGE:
                if Lvec[r] <= l:
                    r += 1
                    continue
                r0 = r
                run_w = 0
                while r < NRANGE and Lvec[r] > l:
                    off_arr[r][l] = so + run_w
                    run_w += int(widths[r])
                    r += 1
                out_entries.append((int(so), int(run_w), int(r0 * rw)))
                so += run_w

    for w in range(NW):
        s_woff[w] = so
        emit_levels(Lmax[w], s_off[w], mm_w[w])
        if so % 2:
            so += 1
    s_woff[NW] = so
    emit_levels(LOmax, so_off, mm_ovf)
    if so % 2:
        so += 1
    s_woff[NW + 1] = so
    SW = int(so)
    # split mm entries at psum bank boundaries (512 fp32 per bank)
    def split_banks(lst):
        out = []
        for (so_, wd, po) in lst:
            while wd > 0:
                room = 512 - (po % 512)
                take = min(wd, room)
                out.append((so_, take, po))
                so_ += take; po += take; wd -= take
        return out
    mm_w = [split_banks(x) for x in mm_w]
    mm_ovf = split_banks(mm_ovf)
    for w in range(NW + 1):
        assert (s_woff[w + 1] - s_woff[w]) <= 2046, f"S win {w} too wide"

    meta = dict(
        nc=nc, csh=csh, sh=sh, npad=npad, cf=cf, dmax=dmax,
        CLS=CLS, XW=XW, SW=SW, F=F, B=B, G=G, NRANGE=NRANGE, rw=rw,
        x0_off=x0_off, x_off=x_off, x0_woff=x0_woff, x_woff=x_woff,
        expand_list=expand_list, m=m, widths=widths,
        Lmax=Lmax, LOmax=LOmax, s_off=s_off, so_off=so_off, s_woff=s_woff,
        mm_w=mm_w, mm_ovf=mm_ovf, G_w=G_w, g_base=g_base,
    )
    per_core = [emit_core_arrays(c, meta) for c in cores_raw]
    return meta, per_core


def emit_core_arrays(c, meta):
    cf, csh = meta["cf"], meta["csh"]
    CLS, F, G = meta["CLS"], meta["F"], meta["G"]
    x0_off, x_off = meta["x0_off"], meta["x_off"]
    x_woff, s_woff = meta["x_woff"], meta["s_woff"]
    s_off, so_off = meta["s_off"], meta["so_off"]
    rw = meta["rw"]

    def put(arr, prt, pos, tgt):
        arr[prt, pos] = tgt.astype(np.int16)

    # class rank of each source within (p, w, d)
    cls_key = (c["usrc_p"] * NW + c["win_of_usrc"]) * (int(c["d_src"].max()) + 1) + c["d_src"]
    cls_rank = _cumcount(cls_key)

    # ls1 split per window-class region (lets expand(w0) overlap ls1b)
    x0w1 = int(meta["x0_woff"][1])
    tgt = x0_off[c["win_of_usrc"], c["d_src"]] + cls_rank
    assert tgt.max() < CLS
    in_a = tgt < x0w1
    ls1a = np.full((P, cf), -1, np.int16)
    put(ls1a, c["usrc_p"][in_a], c["usrc_q"][in_a], tgt[in_a])
    ls1b = np.full((P, cf), -1, np.int16)
    put(ls1b, c["usrc_p"][~in_a], c["usrc_q"][~in_a], tgt[~in_a] - x0w1)

    # X position per edge
    r_in_src = _cumcount(c["src_inv"])
    si = c["src_inv"]
    xpos = x_off[c["w_e"], c["d_src"][si]] + cls_rank[si] * c["d_src"][si] + r_in_src

    ls2 = []
    for w in range(NW):
        wlen = int(x_woff[w + 1] - x_woff[w])
        a2 = np.full((P, wlen), -1, np.int16)
        selm = (c["w_e"] == w) & c["main"]
        xl = xpos[selm] - x_woff[w]
        t2 = (c["b_e"][selm] - w * BLKW) * P + c["j_e"][selm]
        put(a2, c["p_s"][selm], xl, t2)
        om = c["w_e"][c["oi"]] == w          # mask over oi order
        xo = xpos[c["oi"]][om] - x_woff[w]
        to = BPW * P + c["t_o"][om]
        put(a2, c["p_s"][c["oi"]][om], xo, to)
        ls2.append(a2)

    # ls3 (main): input XT[:, w*BLKW*128 : +BPW*128], partition j
    ls3 = []
    r_v = c["c_v"] // rw
    for w in range(NW):
        wlen = BPW * P
        slen = int(s_woff[w + 1] - s_woff[w])
        arr = np.full((P, wlen), -1, np.int16)
        selm = (c["w_e"] == w) & c["main"]
        ipos = (c["b_e"][selm] - w * BLKW) * P + c["p_s"][selm]
        t3 = (
            s_off[w, r_v[selm], c["l_e"][selm]]
            + (c["c_v"][selm] - r_v[selm] * rw)
            - s_woff[w]
        )
        assert len(t3) == 0 or (t3.min() >= 0 and t3.max() < slen)
        put(arr, c["j_e"][selm], ipos, t3)
        ls3.append(arr)

    # lsa call w: input XT ovf block (w*BLKW+BPW) [P, 128] -> XO chunk G_w blocks
    oi = c["oi"]
    lsa = []
    for w in range(NW):
        arr = np.full((P, P), -1, np.int16)
        if len(oi):
            sel = c["w_e"][oi] == w
            ipos = c["p_s"][oi][sel]
            ta = c["g_o"][sel] * P + c["jo"][sel]
            put(arr, c["t_o"][sel], ipos, ta)
        lsa.append(arr)

    # lsb: XOT [P, G*128] -> SM ovf window at (j*, so_off + col)
    g_base = meta["g_base"]
    slen_o = int(s_woff[NW + 1] - s_woff[NW])
    lsb = np.full((P, G * P), -1, np.int16)
    if len(oi) and slen_o:
        gg = g_base[c["w_e"][oi]] + c["g_o"]
        ipos = gg * P + c["t_o"]
        tb = (
            so_off[r_v[oi], c["lo_"]]
            + (c["c_v"][oi] - r_v[oi] * rw)
            - s_woff[NW]
        )
        assert tb.min() >= 0 and tb.max() < slen_o
        put(lsb, c["jo"], ipos, tb)

    return dict(ls1a=ls1a, ls1b=ls1b, ls2=ls2, ls3=ls3, lsa=lsa, lsb=lsb)


# ──────────────────────────────────────────────────────────────────────
# numpy emulation (bf16 values, 1 int16 elem per value)
# ──────────────────────────────────────────────────────────────────────

def _emu_ls(data_i16, idx_i16, num_elems):
    Pp, n = idx_i16.shape
    assert data_i16.shape == (Pp, n)
    out = np.zeros((Pp, num_elems), np.int16)
    for p in range(Pp):
        ii = idx_i16[p].astype(np.int64)
        valid = ii >= 0
        assert len(np.unique(ii[valid])) == valid.sum(), "dup idx"
        out[p, ii[valid]] = data_i16[p, valid]
    return out


def emulate_round(w_full, meta, arrs):
    """w_full: [P, cf] float32 (will be cast bf16). Returns psum [SEGS, csh] f32."""
    cf, csh = meta["cf"], meta["csh"]
    CLS, XW, SW, F, B, G = (meta[k] for k in ("CLS", "XW", "SW", "F", "B", "G"))
    x_woff, s_woff = meta["x_woff"], meta["s_woff"]

    d16 = np.ascontiguousarray(w_full.astype(BF16)).view(np.int16)
    x0w1 = int(meta["x0_woff"][1])
    x0 = np.zeros((P, CLS), np.int16)
    x0[:, 0:x0w1] = _emu_ls(d16, arrs["ls1a"], x0w1)
    x0[:, x0w1:CLS] = _emu_ls(d16, arrs["ls1b"], CLS - x0w1)
    x0 = x0.view(BF16)

    x = np.zeros((P, XW), BF16)
    for (o0, mm, d, o, w) in meta["expand_list"]:
        x[:, o : o + mm * d] = np.repeat(x0[:, o0 : o0 + mm], d, axis=1)

    x2 = np.zeros((P, F), BF16)
    for w in range(NW):
        lo_, hi = int(x_woff[w]), int(x_woff[w + 1])
        seg = np.ascontiguousarray(x[:, lo_:hi]).view(np.int16)
        o = _emu_ls(seg, arrs["ls2"][w], BLKW * P).view(BF16)
        x2[:, w * BLKW * P : (w + 1) * BLKW * P] = o

    xt = np.zeros((P, F), BF16)
    for b in range(B):
        xt[:, b * P : (b + 1) * P] = x2[:, b * P : (b + 1) * P].T

    sm = np.zeros((P, SW), BF16)
    for w in range(NW):
        sl = int(s_woff[w + 1] - s_woff[w])
        seg = np.ascontiguousarray(xt[:, w * BLKW * P : w * BLKW * P + BPW * P]).view(np.int16)
        o = _emu_ls(seg, arrs["ls3"][w], sl).view(BF16)
        sm[:, int(s_woff[w]) : int(s_woff[w]) + sl] = o

    assert int(s_woff[NW + 1] - s_woff[NW]) == 0, "ovf path removed"

    psum = np.zeros((SEGS, csh), np.float32)
    smf = sm.astype(np.float32)
    for lst in (meta["mm_w"][0], meta["mm_w"][1], meta["mm_ovf"]):
        for (so, wd, po) in lst:
            psum[:, po : po + wd] += smf[:, so : so + wd].reshape(SEGS, R4, wd).sum(axis=1)
    return psum


# ─── preprocessing glue ───


def next_pow2(x):
    p = 1
    while p < x:
        p *= 2
    return p


def preprocess(x, edge_index, batch, nc_count=8, n_graphs=128):
    n_nodes = x.shape[0]
    row = np.asarray(edge_index[0], np.int64)
    col = np.asarray(edge_index[1], np.int64)
    batch = np.asarray(batch, np.int64)

    csh, sh, npad, cf = build_layout(n_nodes, nc_count)
    deg = np.bincount(col, minlength=npad).astype(np.int64)
    flat, layout = relabel(deg, n_nodes, nc_count)
    re, ve = flat[row], flat[col]

    cores_raw = []
    for c in range(nc_count):
        m = (ve // sh) == c
        cores_raw.append(build_core(c, re[m], ve[m], layout))
    meta, per_core = finalize_cores(cores_raw, layout)

    # device-order node arrays
    inv = np.empty(npad, np.int64)          # flat -> original id
    inv[flat] = np.arange(npad)
    deg_dev = deg[inv].astype(np.float64)   # deg at device flat position
    batch_dev = np.full(npad, -1, np.int64)
    batch_dev[flat[:n_nodes]] = batch[:n_nodes]

    dinv_dev = np.where(deg_dev > 0, deg_dev ** -0.5, 0.0).astype(np.float32)

    # ---- pooling structures ----
    g0 = np.zeros(nc_count, np.int64)
    ngl = np.zeros(nc_count, np.int64)
    wg_max = 0
    for c in range(nc_count):
        bd = batch_dev[c * sh:(c + 1) * sh]
        real = bd >= 0
        gmin, gmax = (int(bd[real].min()), int(bd[real].max())) if real.any() else (0, 0)
        g0[c], ngl[c] = gmin, gmax - gmin + 1
        fin = np.arange(sh)
        s = fin // csh
        cnt = np.zeros((SEGS, int(ngl[c])), np.int64)
        np.add.at(cnt, (s[real], bd[real] - gmin), 1)
        wg_max = max(wg_max, int(cnt.max()))
    NGLP = int(ngl.max())
    # pool over [128, CP8] layout: partition p = chunk*16 + s, chunk = c // CP8
    CP8 = cdiv(csh, 8)
    pool_idx = []
    gms = [np.zeros((P, P), np.float32) for _ in range(2)]
    w8_max = 0
    ranks = []
    for c in range(nc_count):
        bd = batch_dev[c * sh:(c + 1) * sh]
        fin = np.arange(sh)
        s, cc = fin // csh, fin % csh
        lg = bd - g0[c]
        pp_ = (cc // CP8) * SEGS + s
        pos = cc % CP8
        rank = np.zeros(sh, np.int64)
        real = bd >= 0
        key = pp_ * 4096 + lg
        rank[real] = _cumcount(key[real])
        ranks.append((pp_, pos, lg, rank, real))
        if real.any():
            w8_max = max(w8_max, int(rank[real].max()) + 1)
    W8 = next_pow2(w8_max)
    PH8 = NGLP * W8
    assert PH8 <= 2046, f"pool window {PH8}"
    assert NGLP <= 32
    for c in range(nc_count):
        pp_, pos, lg, rank, real = ranks[c]
        a = np.full((P, CP8), -1, np.int16)
        tgt = lg[real] * W8 + rank[real]
        a[pp_[real], pos[real]] = tgt.astype(np.int16)
        pool_idx.append(a)
        for li in range(NGLP):
            g = g0[c] + li
            if li < int(ngl[c]) and g < n_graphs:
                fp = c * 32 + li
                gms[fp % 2][fp // 2, g] = 1.0

    pool_meta = dict(NGLP=NGLP, W8=W8, PH8=PH8, CP8=CP8, g0=g0)
    return meta, per_core, pool_meta, pool_idx, gms, flat, dinv_dev, layout


def make_inputs(meta, pool_meta, per_core, pool_idx, gms, flat, dinv_dev,
                x, weights, n_max, n_graphs=128):
    """Build per-core in_maps. weights = dict(W1..Wl, b1..bl)."""
    csh, sh, npad, cf = meta["csh"], meta["sh"], meta["npad"], meta["cf"]
    nc_count = meta["nc"]
    n_nodes = x.shape[0]

    # x in device order, transposed: xT_dev[c] = [128, sh], bf16
    xdev = np.zeros((npad, x.shape[1]), np.float32)
    xdev[flat[:n_nodes]] = x
    dinvf = dinv_dev.reshape(P, cf)
    slo = int(meta["s_woff"][NW + 1] - meta["s_woff"][NW])

    wpack = np.zeros((64, 327), np.float32)
    wpack[:, 0:128] = weights["W1"].T
    wpack[:, 128:192] = weights["W2"].T
    wpack[:, 192:256] = weights["W3"].T
    wpack[:, 256:320] = weights["W4"].T
    wpack[:, 320:321] = weights["Wl"].reshape(64, 1)
    for k in range(1, 5):
        wpack[:, 320 + k:321 + k] = np.asarray(weights[f"b{k}"], np.float32).reshape(64, 1)
    wpack[0, 325] = float(np.asarray(weights["bl"]).ravel()[0])
    wpack[0, 326] = 1.0 / np.float32(n_max)
    bfpack = np.zeros((P, 144), np.float32)
    bfpack[:, 0:128] = np.eye(P)
    bfpack[:, 128:144] = np.repeat(np.eye(SEGS), R4, axis=0)
    gmpack = np.concatenate([gms[0], gms[1]], axis=1)

    in_maps = []
    for c in range(nc_count):
        im = dict(
            xT=np.ascontiguousarray(xdev[c * sh:(c + 1) * sh].T).astype(BF16),
            dinvf=dinvf.astype(np.float32),
            dpack=np.concatenate([
                dinv_dev[c * sh:(c + 1) * sh].reshape(SEGS, csh),
                (dinv_dev[c * sh:(c + 1) * sh] ** 2).reshape(SEGS, csh)],
                axis=1).astype(np.float32),
            ls1p=np.concatenate([per_core[c]["ls1a"], per_core[c]["ls1b"]], axis=1),
            ls2p=np.concatenate(per_core[c]["ls2"], axis=1),
            ls3p=np.concatenate(per_core[c]["ls3"], axis=1),
            wpack=wpack,
            bfpack=bfpack.astype(BF16),
            gmpack=gmpack.astype(np.float32),
        )
        if slo > 0:
            im["lsb"] = per_core[c]["lsb"]
            for w in range(NW):
                im[f"lsa_{w}"] = per_core[c]["lsa"][w]
        im["pool8"] = pool_idx[c]
        in_maps.append(im)
    return in_maps


def reference_numpy(x, edge_index, batch, weights, n_graphs=128):
    """Direct numpy reference of the original model."""
    row = np.asarray(edge_index[0]); col = np.asarray(edge_index[1])
    N = x.shape[0]
    deg = np.bincount(col, minlength=N).astype(np.float64)
    dinv = np.where(deg > 0, deg ** -0.5, 0.0)
    norm = dinv[row] * dinv[col]
    h = x.astype(np.float64)
    for k in range(1, 5):
        W = weights[f"W{k}"]
        b = weights[f"b{k}"]
        hw = h @ W
        msg = norm[:, None] * hw[row]
        out = np.zeros((N, hw.shape[1]))
        np.add.at(out, col, msg)
        h = out + b
    sums = np.zeros((n_graphs, h.shape[1]))
    np.add.at(sums, batch, h)
    counts = np.bincount(batch, minlength=n_graphs)
    pooled = sums / counts.max()
    return (pooled @ weights["Wl"] + weights["bl"]).astype(np.float32)


# ─── device kernel ───
from contextlib import ExitStack

import concourse.bass as bass
import concourse.tile as tile
from concourse import bacc, mybir

FP32 = mybir.dt.float32
BF = mybir.dt.bfloat16
I16 = mybir.dt.int16
AT = mybir.ActivationFunctionType
OP = mybir.AluOpType


def build_kernel(meta, pool_meta, n_graphs=128):
    csh, sh, npad, cf = meta["csh"], meta["sh"], meta["npad"], meta["cf"]
    CLS, XW, SW, F, B, G = (meta[k] for k in ("CLS", "XW", "SW", "F", "B", "G"))
    x0_woff, x_woff, s_woff = meta["x0_woff"], meta["x_woff"], meta["s_woff"]
    G_w, g_base = meta["G_w"], meta["g_base"]
    mm_w, mm_ovf = meta["mm_w"], meta["mm_ovf"]
    NGLP, W8 = pool_meta["NGLP"], pool_meta["W8"]
    PH8, CP8 = pool_meta["PH8"], pool_meta["CP8"]
    core_ids = list(range(meta["nc"]))

    nc = bacc.Bacc("TRN2", target_bir_lowering=False, debug=False,
                   num_devices=meta["nc"])

    def din(name, shape, dt=FP32):
        return nc.declare_dram_parameter(name, list(shape), dt, isOutput=False)

    slo_pre = int(s_woff[NW + 1] - s_woff[NW])
    xw_tot = int(x_woff[NW])
    # ---- inputs ----
    xT_in = din("xT", [P, sh], BF)
    dinvf_in = din("dinvf", [P, cf])
    dpack_in = din("dpack", [SEGS, 2 * csh])
    ls1p_in = din("ls1p", [P, 2 * cf], I16)
    ls2p_in = din("ls2p", [P, xw_tot], I16)
    ls3p_in = din("ls3p", [P, NW * BPW * P], I16)
    if slo_pre > 0:
        lsa_in = [din(f"lsa_{w}", [P, P], I16) for w in range(NW)]
        lsb_in = din("lsb", [P, G * P], I16)
    pool8_in = din("pool8", [P, CP8], I16)
    wpack_in = din("wpack", [64, 327])
    bfpack_in = din("bfpack", [P, 144], BF)
    gmpack_in = din("gmpack", [P, 256])
    out_ext = nc.declare_dram_parameter("out", [n_graphs], FP32, isOutput=True)

    # ---- internal DRAM ----
    sh_dram = nc.dram_tensor("sh_dram", [sh], BF)
    full_dram = nc.dram_tensor("full_dram", [npad], BF, addr_space="Shared")
    part_dram = nc.dram_tensor("part_dram", [32], FP32)
    warm_in = nc.dram_tensor("warm_in", [32], FP32)
    warm_out = nc.dram_tensor("warm_out", [256], FP32, addr_space="Shared")
    partall_dram = nc.dram_tensor("partall_dram", [256], FP32, addr_space="Shared")

    slo = int(s_woff[NW + 1] - s_woff[NW])

    with tile.TileContext(nc) as tc:
        with ExitStack() as ctx:
            pool = ctx.enter_context(tc.tile_pool(name="p", bufs=1))
            tp = ctx.enter_context(tc.tile_pool(name="tp", bufs=3, space="PSUM"))
            up = ctx.enter_context(tc.tile_pool(name="up", bufs=1, space="PSUM"))

            # persistent tiles
            state = pool.tile([P, cf], BF)
            wbuf = pool.tile([P, cf], BF)
            tbuf = pool.tile([P, cf], FP32)
            dinvf = pool.tile([P, cf], FP32)
            dpk = pool.tile([SEGS, 2 * csh], FP32)
            dinvs = dpk[:, 0:csh]
            dinv2s = dpk[:, csh:2 * csh]
            bdfs = pool.tile([SEGS, csh], FP32)
            x0 = pool.tile([P, CLS], BF)
            xbuf = pool.tile([P, XW], BF)
            x2 = [pool.tile([P, BLKW * P], BF, name=f"x2_{w}") for w in range(NW)]
            xt = [pool.tile([P, BLKW * P], BF, name=f"xt_{w}") for w in range(NW)]
            sm = pool.tile([P, SW], BF)
            xo = pool.tile([P, G * P], BF)
            xot = pool.tile([P, G * P], BF)
            u_bf = pool.tile([SEGS, csh], BF)
            s4_bf = pool.tile([SEGS, CP8 * 8], BF)
            s4r = pool.tile([P, CP8], BF)
            t4 = pool.tile([SEGS, csh], FP32)
            bfp = pool.tile([P, 144], BF)
            ident = bfp[:, 0:128]
            sel = bfp[:, 128:144]
            ones16 = pool.tile([SEGS, 1], FP32)
            ones128 = pool.tile([1, P], FP32)
            gmp = pool.tile([P, 256], FP32)
            gm = [gmp[:, 0:128], gmp[:, 128:256]]
            ls1p = pool.tile([P, 2 * cf], I16)
            ls2p = pool.tile([P, xw_tot], I16)
            ls2 = [ls2p[:, int(x_woff[w]):int(x_woff[w + 1])] for w in range(NW)]
            ls3p = pool.tile([P, NW * BPW * P], I16)
            ls3 = [ls3p[:, w * BPW * P:(w + 1) * BPW * P] for w in range(NW)]
            if slo_pre > 0:
                lsa = [pool.tile([P, P], I16, name=f"lsat{w}") for w in range(NW)]
                lsb = pool.tile([P, G * P], I16)
            plidx8 = pool.tile([P, CP8], I16)
            poolb8 = pool.tile([P, PH8], BF)
            poolf8 = pool.tile([P, PH8], FP32)
            ones128c = pool.tile([P, 1], FP32)
            part_sb = pool.tile([1, 32], FP32)
            partall = pool.tile([P, 2], FP32)
            outrow = pool.tile([1, n_graphs], FP32)
            stage = pool.tile([1, sh], BF)
            wpk = pool.tile([64, 327], FP32)
            wts = {
                "w1t": wpk[:, 0:128],
                "w2t": wpk[:, 128:192],
                "w3t": wpk[:, 192:256],
                "w4t": wpk[:, 256:320],
                "wl": wpk[:, 320:321],
            }
            bs = [wpk[:, 321 + k:322 + k] for k in range(4)]
            blt = wpk[0:1, 325:326]
            invn = wpk[0:1, 326:327]
            cvec = {
                "c3": pool.tile([64, 1], FP32, name="c3t"),
                "c2": pool.tile([64, 1], FP32, name="c2t"),
                "c1": pool.tile([64, 1], FP32, name="c1t"),
                "c0": pool.tile([128, 1], FP32, name="c0t"),
            }
            c0b = pool.tile([128, 1], BF)
            betas = pool.tile([1, 4], FP32)
            betas16 = pool.tile([SEGS, 4], FP32)

            # ---- warmup collective + early weight load ----
            warmsb = pool.tile([1, 32], FP32)
            nc.vector.memset(warmsb[:], 0.0)
            nc.vector.memset(ones16[:], 1.0)
            nc.vector.memset(ones128[:], 1.0)
            nc.vector.memset(ones128c[:], 1.0)
            if CP8 * 8 > csh:
                nc.vector.memset(s4_bf[:, csh:CP8 * 8], 0.0)
            nc.sync.dma_start(warm_in[:].rearrange("(a b) -> a b", a=1),
                              warmsb[:])
            nc.sync.dma_start(wpk[:], wpack_in[:])
            nc.gpsimd.collective_compute(
                "AllGather", OP.bypass, replica_groups=[core_ids],
                ins=[warm_in[:]], outs=[warm_out[:]],
            )

            # ---- c chain + betas ----
            pc = tp.tile([128, 8], FP32, tag="ptr")
            nc.tensor.matmul(pc[0:64, 0:1], wts["w4t"][:], wts["wl"][:],
                             start=True, stop=True)
            nc.vector.tensor_copy(cvec["c3"][:], pc[0:64, 0:1])
            nc.tensor.matmul(pc[0:64, 1:2], wts["w3t"][:], cvec["c3"][:],
                             start=True, stop=True)
            nc.vector.tensor_copy(cvec["c2"][:], pc[0:64, 1:2])
            nc.tensor.matmul(pc[0:64, 2:3], wts["w2t"][:], cvec["c2"][:],
                             start=True, stop=True)
            nc.vector.tensor_copy(cvec["c1"][:], pc[0:64, 2:3])
            nc.tensor.matmul(pc[0:128, 3:4], wts["w1t"][:], cvec["c1"][:],
                             start=True, stop=True)
            nc.vector.tensor_copy(cvec["c0"][:], pc[0:128, 3:4])
            nc.vector.tensor_copy(c0b[:], cvec["c0"][:])
            pb = tp.tile([1, 4], FP32, tag="ptr")
            for k, cn in enumerate(["c1", "c2", "c3"]):
                nc.tensor.matmul(pb[0:1, k:k + 1], bs[k][:], cvec[cn][:],
                                 start=True, stop=True)
            nc.tensor.matmul(pb[0:1, 3:4], bs[3][:], wts["wl"][:],
                             start=True, stop=True)
            nc.vector.tensor_copy(betas[:], pb[:])
            pbb = tp.tile([P, 4], FP32, tag="ptr")
            nc.tensor.matmul(pbb[:], ones128[:], betas[:], start=True, stop=True)
            nc.vector.tensor_copy(betas16[:], pbb[0:SEGS, :])

            # ---- s0 = x @ c0 (bf16, 8 chunks, pipelined) ----
            NCH = 16
            chw = sh // NCH
            assert chw % 2 == 0
            pw2 = chw // 2
            qeng = [nc.sync, nc.scalar, nc.gpsimd]
            xfull = pool.tile([P, sh], BF)
            for q in range(NCH):
                qeng[q % 3].dma_start(xfull[:, q * chw:(q + 1) * chw],
                                      xT_in[:, q * chw:(q + 1) * chw])
            shv = sh_dram[:].rearrange("(a b) -> a b", a=1)
            for q in range(NCH):
                for pi in range(2):
                    ps0 = tp.tile([P, 512], FP32, tag="ptr", name=f"ps0_{q}_{pi}")
                    nc.tensor.matmul(
                        ps0[0:1, 0:pw2], c0b[:],
                        xfull[:, q * chw + pi * pw2: q * chw + (pi + 1) * pw2],
                        start=True, stop=True)
                    eng = nc.vector if (q + pi) % 2 == 0 else nc.scalar
                    if eng is nc.vector:
                        nc.vector.tensor_copy(
                            stage[:, q * chw + pi * pw2: q * chw + (pi + 1) * pw2],
                            ps0[0:1, 0:pw2])
                    else:
                        nc.scalar.activation(
                            stage[:, q * chw + pi * pw2: q * chw + (pi + 1) * pw2],
                            ps0[0:1, 0:pw2], AT.Copy)
                qeng[q % 3].dma_start(
                    shv[:, q * chw:(q + 1) * chw], stage[:, q * chw:(q + 1) * chw])

            # ---- index-table loads, issue spread across engine queues ----
            nc.scalar.dma_start(ls1p[:], ls1p_in[:])
            nc.gpsimd.dma_start(ls2p[:], ls2p_in[:])
            nc.sync.dma_start(ls3p[:], ls3p_in[:])
            nc.scalar.dma_start(dinvf[:], dinvf_in[:])
            nc.gpsimd.dma_start(dpk[:], dpack_in[:])
            nc.sync.dma_start(bfp[:], bfpack_in[:])
            nc.scalar.dma_start(gmp[:], gmpack_in[:])
            if slo_pre > 0:
                for w in range(NW):
                    nc.sync.dma_start(lsa[w][:], lsa_in[w][:])
                nc.scalar.dma_start(lsb[:], lsb_in[:])
            nc.gpsimd.dma_start(plidx8[:], pool8_in[:])

            # ---- state rounds ----
            def allgather_state(dst):
                nc.gpsimd.collective_compute(
                    "AllGather", OP.bypass, replica_groups=[core_ids],
                    ins=[sh_dram[:]], outs=[full_dram[:]],
                )
                nc.sync.dma_start(
                    dst[:], full_dram[:].rearrange("(p c) -> p c", p=P))

            allgather_state(state)

            # PSUM accumulation is bank-scoped: the first matmul into a bank
            # (start=True) clears the bank's has_written bits; later matmuls
            # (start=False) overwrite-on-first-touch / accumulate-where-set
            # per element. So per round emit exactly one start and one stop
            # per bank, regardless of region interleaving.
            bank_total = {}
            for lst in (mm_w + [mm_ovf]):
                for (so, wd, po) in lst:
                    b = po // 512
                    assert (po + wd - 1) // 512 == b
                    bank_total[b] = bank_total.get(b, 0) + 1

            for rnd in range(4):
                if rnd == 0:
                    # w = bf16(state * dinv) in fp32 (receiver side, rnd 0 only)
                    nc.vector.tensor_copy(tbuf[:], state[:])
                    nc.vector.tensor_tensor(wbuf[:], tbuf[:], dinvf[:], OP.mult)
                # LS1 (two calls: w0 classes then w1 classes, so the w0
                # expand overlaps the second call)
                x0w1 = int(x0_woff[1])
                nc.gpsimd.local_scatter(
                    x0[:, 0:x0w1].bitcast(I16), wbuf[:].bitcast(I16),
                    ls1p[:, 0:cf], channels=P, num_elems=x0w1, num_idxs=cf)
                nc.gpsimd.local_scatter(
                    x0[:, x0w1:CLS].bitcast(I16), wbuf[:].bitcast(I16),
                    ls1p[:, cf:2 * cf], channels=P, num_elems=CLS - x0w1,
                    num_idxs=cf)
                if rnd < 3:
                    # bdfs for this round's output state (off critical path)
                    nc.vector.tensor_scalar(
                        bdfs[:], dinvs[:], betas16[:, rnd:rnd + 1], None, OP.mult)
                # expand + LS2 per window
                for w in range(NW):
                    ei = 0
                    for (o0, mm_, d, o, we) in meta["expand_list"]:
                        if we != w:
                            continue
                        src = x0[:, o0:o0 + mm_].unsqueeze(2).broadcast_to([P, mm_, d])
                        dst = xbuf[:, o:o + mm_ * d].rearrange("p (m d) -> p m d", d=d)
                        if ei % 2 == 0:
                            nc.vector.tensor_copy(dst, src)
                        else:
                            nc.scalar.activation(dst, src, AT.Copy)
                        ei += 1
                    lo_, hi = int(x_woff[w]), int(x_woff[w + 1])
                    nc.gpsimd.local_scatter(
                        x2[w][:].bitcast(I16),
                        xbuf[:, lo_:hi].bitcast(I16), ls2[w][:],
                        channels=P, num_elems=BLKW * P,
                        num_idxs=(hi - lo_))
                    # transposes of this window's blocks (PE, overlaps next LS2)
                    for b0 in range(0, BLKW, 4):
                        nb = min(4, BLKW - b0)
                        pt = tp.tile([P, 512], BF, tag="ptr", name=f"pt{rnd}_{w}_{b0}")
                        for k in range(nb):
                            b = b0 + k
                            nc.tensor.transpose(pt[:, k * P:(k + 1) * P],
                                                x2[w][:, b * P:(b + 1) * P], ident[:])
                        if (b0 // 4) % 2 == 0:
                            nc.vector.tensor_copy(xt[w][:, b0 * P:(b0 + nb) * P],
                                                  pt[:, 0:nb * P])
                        else:
                            nc.scalar.activation(xt[w][:, b0 * P:(b0 + nb) * P],
                                                 pt[:, 0:nb * P], AT.Copy)
                # LS3 + LSA per window; all sel matmuls accumulate into ONE pu
                pu = up.tile([SEGS, csh], FP32, tag="pu", name=f"pu{rnd}")
                bank_seen = {}
                def emit_mms(lst):
                    for (so, wd, po) in lst:
                        b = po // 512
                        seen = bank_seen.get(b, 0)
                        bank_seen[b] = seen + 1
                        nc.tensor.matmul(
                            pu[:, po:po + wd], sel[:], sm[:, so:so + wd],
                            start=(seen == 0),
                            stop=(seen + 1 == bank_total[b]),
                            skip_group_check=True)
                for w in range(NW):
                    sl = int(s_woff[w + 1] - s_woff[w])
                    nc.gpsimd.local_scatter(
                        sm[:, int(s_woff[w]):int(s_woff[w]) + sl].bitcast(I16),
                        xt[w][:].bitcast(I16),
                        ls3[w][:], channels=P, num_elems=sl,
                        num_idxs=BPW * P)
                    if slo > 0:
                        gw = int(G_w[w])
                        gb = int(g_base[w])
                        nc.gpsimd.local_scatter(
                            xo[:, gb * P:(gb + gw) * P].bitcast(I16),
                            xt[w][:, BPW * P:(BPW + 1) * P].bitcast(I16),
                            lsa[w][:], channels=P, num_elems=gw * P,
                            num_idxs=P)
                        # XO transposes for this window's g-blocks (PE)
                        assert gw <= 4
                        pt = tp.tile([P, 512], BF, tag="ptr", name=f"po{rnd}_{w}")
                        for k in range(gw):
                            g = gb + k
                            nc.tensor.transpose(pt[:, k * P:(k + 1) * P],
                                                xo[:, g * P:(g + 1) * P], ident[:])
                        nc.scalar.activation(xot[:, gb * P:(gb + gw) * P],
                                             pt[:, 0:gw * P], AT.Copy)
                    if w == NW - 1:
                        emit_mms([e for e in mm_w[w] if e[2] < 512]
                                 + [e for e in mm_w[w] if e[2] >= 512])
                    else:
                        emit_mms(mm_w[w])
                # ovf: LSB + ovf matmuls
                if slo > 0:
                    nc.gpsimd.local_scatter(
                        sm[:, int(s_woff[NW]):int(s_woff[NW]) + slo].bitcast(I16),
                        xot[:].bitcast(I16), lsb[:],
                        channels=P, num_elems=slo, num_idxs=G * P)
                    emit_mms(mm_ovf)
                assert bank_seen == bank_total

                # sender-side state math: w_next = bf16(u*dinv^2 + beta*dinv)
                # split per psum bank so bank A's math/send overlaps bank B's
                # matmuls.
                if rnd < 3:
                    shv2 = sh_dram[:].rearrange("(s c) -> s c", s=SEGS)
                    for (a, b2) in ((0, 512), (512, csh)):
                        nc.vector.tensor_tensor(
                            t4[:, a:b2], pu[:, a:b2], dinv2s[:, a:b2], OP.mult)
                        nc.vector.tensor_tensor(
                            u_bf[:, a:b2], t4[:, a:b2], bdfs[:, a:b2], OP.add)
                        nc.sync.dma_start(shv2[:, a:b2], u_bf[:, a:b2])
                    allgather_state(wbuf)
                else:
                    nc.vector.tensor_tensor(t4[:], pu[:], dinvs[:], OP.mult)
                    nc.vector.tensor_scalar(
                        s4_bf[:, 0:csh], t4[:], betas16[:, 3:4], None, OP.add)

            # ---- pooling (on [128, CP8] reshaped shard) ----
            for k in range(8):
                qeng[k % 3].dma_start(s4r[k * SEGS:(k + 1) * SEGS, :],
                                      s4_bf[:, k * CP8:(k + 1) * CP8])
            nc.gpsimd.local_scatter(
                poolb8[:].bitcast(I16), s4r[:].bitcast(I16), plidx8[:],
                channels=P, num_elems=PH8, num_idxs=CP8)
            nc.vector.tensor_copy(poolf8[:], poolb8[:])
            wgp = W8
            a = poolf8[:].rearrange("p (g t) -> p g t", t=W8)
            while wgp > 1:
                hw = wgp // 2
                nc.vector.tensor_tensor(
                    a[:, :, 0:hw], a[:, :, 0:hw], a[:, :, hw:wgp], OP.add)
                wgp = hw
            pp = tp.tile([1, 512], FP32, tag="ptr")
            nc.tensor.matmul(pp[0:1, 0:NGLP], ones128c[:], a[:, :, 0],
                             start=True, stop=True)
            nc.vector.memset(part_sb[:], 0.0)
            nc.vector.tensor_copy(part_sb[:, 0:NGLP], pp[0:1, 0:NGLP])
            nc.sync.dma_start(part_dram[:].rearrange("(a b) -> a b", a=1),
                              part_sb[:])
            nc.gpsimd.collective_compute(
                "AllGather", OP.bypass, replica_groups=[core_ids],
                ins=[part_dram[:]], outs=[partall_dram[:]],
            )
            nc.sync.dma_start(partall[:],
                              partall_dram[:].rearrange("(p c) -> p c", p=P))
            po_ = tp.tile([1, n_graphs], FP32, tag="ptr")
            nc.tensor.matmul(po_[:], partall[:, 0:1], gm[0][:],
                             start=True, stop=False)
            nc.tensor.matmul(po_[:], partall[:, 1:2], gm[1][:],
                             start=False, stop=True)
            nc.vector.tensor_copy(outrow[:], po_[:])
            nc.vector.tensor_scalar(outrow[:], outrow[:], invn[0:1, 0:1],
                                    None, OP.mult)
            nc.vector.tensor_scalar(outrow[:], outrow[:], blt[0:1, 0:1],
                                    None, OP.add)
            nc.sync.dma_start(out_ext[:].rearrange("(a b) -> a b", a=1),
                              outrow[:])
    return nc


# ─── entry point ───

def kernel(x, edge_index, batch, W1, b1, W2, b2, W3, b3, W4, b4, Wl, bl):
    from concourse.bass_utils import run_bass_kernel_spmd

    x = np.asarray(x, np.float32)
    edge_index = np.asarray(edge_index)
    batch = np.asarray(batch)
    weights = dict(W1=np.asarray(W1, np.float32), W2=np.asarray(W2, np.float32),
                   W3=np.asarray(W3, np.float32), W4=np.asarray(W4, np.float32),
                   Wl=np.asarray(Wl, np.float32),
                   b1=np.asarray(b1, np.float32), b2=np.asarray(b2, np.float32),
                   b3=np.asarray(b3, np.float32), b4=np.asarray(b4, np.float32),
                   bl=np.asarray(bl, np.float32))
    n_graphs = 128

    meta, per_core, pool_meta, pool_idx, gms, flat, dinv_dev, layout = \
        preprocess(x, edge_index, batch, 8, n_graphs)
    n_max = int(np.bincount(np.asarray(batch, np.int64),
                            minlength=n_graphs).max())
    in_maps = make_inputs(meta, pool_meta, per_core, pool_idx, gms, flat,
                          dinv_dev, x, weights, n_max, n_graphs)
    nc = build_kernel(meta, pool_meta, n_graphs)
    nc.finalize()
    res = run_bass_kernel_spmd(nc, in_maps, core_ids=list(range(8)),
                               trace=False)
    return res.results[0]["out"].reshape(n_graphs, 1).astype(np.float32)


# revision 24
# speedup vs baseline: 1.0690x; 1.0104x over previous
"""Trainium2 kernel for nn_GCNRegression: linear-GCN scalar collapse, bf16 edge pipeline.

The model is linear (no activation), so 4 GCN layers + mean-pool +
linear head collapse exactly to scalar propagation through the graph:
    c0 = W1 @ W2 @ W3 @ W4 @ Wl;  s0 = x @ c0
    s_k = dinv * (Adj @ (dinv * s_{k-1})) + b_k . c_k
    out[g] = sum_{v in g} s4[v] / n_max + bl
Runs on 8 NeuronCores. Per round: AllGather bf16 state, gpsimd
local_scatter routing (bf16 streams), PE transposes, PE segment
reduction accumulating fp32 in PSUM. All index arrays are
host-precomputed from the edge list.
"""

import sys

sys.path.insert(0, "/opt/trn_rl_repo")

import numpy as np
import ml_dtypes

BF16 = ml_dtypes.bfloat16

P = 128          # partitions
SEGS = 16        # shard rows (psum partitions)
R4 = P // SEGS   # 8 rows per segment
NW = 2           # windows (= LS2/LS3 call count)
BPW = 10         # main blocks per window
BLKW = BPW       # no ovf block (CAP chosen so nothing spills)
B_TOT = NW * BLKW  # total X2/XT blocks
CAP = R4 * BPW   # capacity per (p_s, w, s_v) cell
NRANGE = 8
SBUDGET = 2046   # bf16 values per S window (local_scatter num_elems limit)
CLS_MAX = 2046


def cdiv(a, b):
    return (a + b - 1) // b


def _cumcount(keys):
    """Rank of each element within its key group (stable, array order)."""
    order = np.argsort(keys, kind="stable")
    sk = keys[order]
    grp_start = np.r_[0, np.flatnonzero(sk[1:] != sk[:-1]) + 1]
    sizes = np.diff(np.r_[grp_start, len(keys)])
    cum = np.arange(len(keys)) - np.repeat(grp_start, sizes)
    out = np.empty(len(keys), np.int64)
    out[order] = cum
    return out


def build_layout(n_nodes, nc):
    csh = cdiv(n_nodes, nc * SEGS)
    sh = SEGS * csh
    npad = nc * sh
    cf = npad // P
    return csh, sh, npad, cf


def relabel(edge_col_deg_src, n_nodes, nc):
    """Shard by original id; within shard sort by in-degree desc; lay
    column-major into [SEGS, CSH]. Returns flat[] over padded ids."""
    deg = edge_col_deg_src
    csh, sh, npad, cf = build_layout(n_nodes, nc)
    flat = np.empty(npad, np.int64)
    for c in range(nc):
        ids = np.arange(c * sh, (c + 1) * sh)
        order = np.argsort(-deg[ids], kind="stable")
        t = np.empty(len(ids), np.int64)
        t[order] = np.arange(len(ids))
        s, cc = t % SEGS, t // SEGS
        flat[ids] = c * sh + s * csh + cc
    return flat, (csh, sh, npad, cf)


def build_core(core, re, ve, layout):
    """Per-core assignment. re/ve: device-flat src/dst positions."""
    csh, sh, npad, cf = layout
    E = len(re)
    p_s = re // cf
    fin = ve - core * sh
    s_v = fin // csh
    c_v = fin % csh

    # ---- window per source ----
    usrc, src_inv, src_cnt = np.unique(re, return_inverse=True, return_counts=True)
    usrc_p = usrc // cf
    so = np.lexsort((-src_cnt, usrc_p))
    rank_in_p = _cumcount(usrc_p[so])
    win_of_usrc = np.empty(len(usrc), np.int64)
    win_of_usrc[so] = rank_in_p % NW
    w_e = win_of_usrc[src_inv]

    # ---- overflow: cap (p_s, w, s_v) cells at CAP ----
    cell = (p_s * NW + w_e) * SEGS + s_v
    crank = _cumcount(cell)
    is_ovf = crank >= CAP

    main = ~is_ovf
    # ---- j for main edges ----
    j_e = np.full(E, -1, np.int64)
    mi = np.flatnonzero(main)
    cnt_vw = _cumcount((ve[mi] * NW + w_e[mi]))
    j_e[mi] = s_v[mi] * R4 + (cnt_vw % R4)

    def psj(idx):
        return (p_s[idx] * NW + w_e[idx]) * P + j_e[idx]

    vwj = {}
    def vwj_key(i, jv):
        return (int(ve[i]) * NW + int(w_e[i])) * P + int(jv)
    for _try in range(300):
        k = psj(mi)
        cnt = np.bincount(k, minlength=P * NW * P)
        rank = _cumcount(k)
        move = np.flatnonzero(rank >= BPW)
        if len(move) == 0:
            break
        if _try == 0:
            vk = (ve[mi] * NW + w_e[mi]) * P + j_e[mi]
            uk, uc = np.unique(vk, return_counts=True)
            vwj = dict(zip(uk.tolist(), uc.tolist()))
        for ii in move:
            i = mi[ii]
            base = s_v[i] * R4
            pw = (p_s[i] * NW + w_e[i]) * P
            best = None
            for r in range(R4):
                jv = base + r
                if jv == j_e[i]:
                    continue
                ld = cnt[pw + jv]
                nv = vwj.get(vwj_key(i, jv), 0)
                key = (nv, ld)
                if ld < BPW and (best is None or key < best[0]):
                    best = (key, jv)
            if best is None:
                loads = [cnt[pw + base + r] for r in range(R4)]
                jv = base + int(np.argmin(loads))
            else:
                jv = best[1]
            vwj[vwj_key(i, j_e[i])] = vwj.get(vwj_key(i, j_e[i]), 1) - 1
            cnt[pw + j_e[i]] -= 1
            j_e[i] = jv
            cnt[pw + jv] += 1
            vwj[vwj_key(i, jv)] = vwj.get(vwj_key(i, jv), 0) + 1
    else:
        raise RuntimeError("j balance failed")
    k = psj(mi)
    assert np.bincount(k, minlength=P * NW * P).max() <= BPW

    # ---- b for main ----
    b_e = np.full(E, -1, np.int64)
    b_e[mi] = w_e[mi] * BLKW + _cumcount(k)

    # ---- main layers: rank within (v, w, j) ----
    l_e = np.full(E, -1, np.int64)
    l_e[mi] = _cumcount((ve[mi] * NW + w_e[mi]) * P + j_e[mi])

    return dict(
        E=E, ve=ve, p_s=p_s, s_v=s_v, c_v=c_v, w_e=w_e, j_e=j_e, b_e=b_e,
        l_e=l_e, main=main, is_ovf_cap=is_ovf,
        usrc=usrc, usrc_p=usrc_p, usrc_q=usrc % cf, d_src=src_cnt,
        win_of_usrc=win_of_usrc, src_inv=src_inv,
    )


def assign_ovf(c, Lcap, rw):
    """Phase B: final overflow set = capacity spills + layer spills; assign
    dense ovf slots (t_o unique per (p_s,w); (t_o,jo,g) collision-free with
    small g). Mutates c."""
    E = c["E"]
    r_v = c["c_v"] // rw
    spill_l = np.zeros(E, bool)
    mi = np.flatnonzero(c["main"])
    spill_l[mi] = c["l_e"][mi] >= Lcap[c["w_e"][mi], r_v[mi]]
    is_ovf = c["is_ovf_cap"] | spill_l
    c["main"] = ~is_ovf
    c["oi"] = oi = np.flatnonzero(is_ovf)
    ve, s_v, p_s, w_e = c["ve"], c["s_v"], c["p_s"], c["w_e"]

    jo = np.zeros(len(oi), np.int64)
    t_o = np.zeros(len(oi), np.int64)
    g_o = np.zeros(len(oi), np.int64)
    # greedy dense assignment, per window
    GMAX = 8
    for w in range(NW):
        sel = np.flatnonzero(w_e[oi] == w)
        used_t = np.zeros((P, P), bool)        # (p_s, t) used
        used_cell = np.zeros((GMAX, P, P), bool)   # (g, t, j) used
        for ii in sel:
            i = oi[ii]
            ps = int(p_s[i]); sv = int(s_v[i])
            placed = False
            for g in range(GMAX):
                for r in range(R4):
                    j = sv * R4 + r
                    # scan t options not used for this (p_s)
                    for t in range(P):
                        tt = (t * 37 + ps * 13 + j * 29) % P
                        if used_t[ps, tt] or used_cell[g, tt, j]:
                            continue
                        used_t[ps, tt] = True
                        used_cell[g, tt, j] = True
                        jo[ii], t_o[ii], g_o[ii] = j, tt, g
                        placed = True
                        break
                    if placed:
                        break
                if placed:
                    break
            assert placed, "ovf assignment failed"
    c["jo"] = jo
    c["t_o"] = t_o
    c["g_o"] = g_o
    c["lo_"] = _cumcount(ve[oi] * P + jo)


def finalize_cores(cores_raw, layout):
    csh, sh, npad, cf = layout
    nc = len(cores_raw)
    dmax = max(int(c["d_src"].max()) for c in cores_raw)

    # class sizes m[w][d]: max over (core, partition)
    m = np.zeros((NW, dmax + 1), np.int64)
    for c in cores_raw:
        cnt = np.zeros((P, NW, dmax + 1), np.int64)
        np.add.at(cnt, (c["usrc_p"], c["win_of_usrc"], c["d_src"]), 1)
        m = np.maximum(m, cnt.max(axis=0))
    m[:, 0] = 0

    x0_off = np.zeros((NW, dmax + 1), np.int64)
    x_off = np.zeros((NW, dmax + 1), np.int64)
    x0_woff = np.zeros(NW + 1, np.int64)
    x_woff = np.zeros(NW + 1, np.int64)
    o0 = o = 0
    expand_list = []
    for w in range(NW):
        x0_woff[w] = o0
        x_woff[w] = o
        for d in range(1, dmax + 1):
            if m[w][d] == 0:
                continue
            x0_off[w][d] = o0
            x_off[w][d] = o
            expand_list.append((int(o0), int(m[w][d]), d, int(o), w))
            o0 += int(m[w][d])
            o += int(m[w][d]) * d
        if o0 % 2:      # keep window boundaries even
            o0 += 1
        if o % 2:
            o += 1
    x0_woff[NW] = o0
    x_woff[NW] = o
    CLS, XW = int(o0), int(o)
    assert CLS <= CLS_MAX, f"CLS={CLS}"

    B = B_TOT            # total X2 blocks (BPW main + 1 ovf per window)
    F = B * P

    # main layer counts per (w, col-range); cap so each S window fits
    rw = cdiv(csh, NRANGE)
    widths = [min(rw, csh - r * rw) for r in range(NRANGE)]
    Lmax = np.zeros((NW, NRANGE), np.int64)
    for c in cores_raw:
        mm_ = c["main"]
        r_v = c["c_v"] // rw
        np.maximum.at(Lmax, (c["w_e"][mm_], r_v[mm_]), c["l_e"][mm_] + 1)
    wa = np.array(widths)
    for w in range(NW):
        while int((Lmax[w] * wa).sum()) > SBUDGET:
            r = int(np.argmax(Lmax[w] * 10000 + wa))
            assert Lmax[w][r] > 1, "cannot fit S window"
            Lmax[w][r] -= 1
    for c in cores_raw:
        assign_ovf(c, Lmax, rw)
        assert len(c["oi"]) == 0, f"overflow edges present: {len(c['oi'])}"
    LOmax = np.zeros(NRANGE, np.int64)
    for c in cores_raw:
        r_v = c["c_v"] // rw
        if len(c["oi"]):
            np.maximum.at(LOmax, r_v[c["oi"]], c["lo_"] + 1)
    assert int((LOmax * wa).sum()) <= SBUDGET, f"ovf S window: {int((LOmax*wa).sum())}"

    G_w = np.ones(NW, np.int64)
    for c in cores_raw:
        if len(c["oi"]):
            np.maximum.at(G_w, c["w_e"][c["oi"]], c["g_o"] + 1)
    assert G_w.max() <= 15, f"G_w={G_w}"
    g_base = np.r_[0, np.cumsum(G_w)]
    G = int(g_base[-1])

    # level-major layout: level l of window w holds the contiguous runs of
    # ranges with Lmax > l, so each (w, l, run) is ONE wide matmul.
    s_off = np.zeros((NW, NRANGE, int(Lmax.max() or 1)), np.int64)
    so_off = np.zeros((NRANGE, int(LOmax.max() or 1)), np.int64)
    s_woff = np.zeros(NW + 2, np.int64)
    so = 0
    mm_w = [[] for _ in range(NW)]      # per-window matmul entries
    mm_ovf = []

    def emit_levels(Lvec, off_arr, out_entries):
        nonlocal so
        for l in range(int(Lvec.max()) if len(Lvec) else 0):
            r = 0
            while r < NRANGE:
                if Lvec[r] <= l:
                    r += 1
                    continue
                r0 = r
                run_w = 0
                while r < NRANGE and Lvec[r] > l:
                    off_arr[r][l] = so + run_w
                    run_w += int(widths[r])
                    r += 1
                out_entries.append((int(so), int(run_w), int(r0 * rw)))
                so += run_w

    for w in range(NW):
        s_woff[w] = so
        emit_levels(Lmax[w], s_off[w], mm_w[w])
        if so % 2:
            so += 1
    s_woff[NW] = so
    emit_levels(LOmax, so_off, mm_ovf)
    if so % 2:
        so += 1
    s_woff[NW + 1] = so
    SW = int(so)
    # split mm entries at psum bank boundaries (512 fp32 per bank)
    def split_banks(lst):
        out = []
        for (so_, wd, po) in lst:
            while wd > 0:
                room = 512 - (po % 512)
                take = min(wd, room)
                out.append((so_, take, po))
                so_ += take; po += take; wd -= take
        return out
    mm_w = [split_banks(x) for x in mm_w]
    mm_ovf = split_banks(mm_ovf)
    for w in range(NW + 1):
        assert (s_woff[w + 1] - s_woff[w]) <= 2046, f"S win {w} too wide"

    meta = dict(
        nc=nc, csh=csh, sh=sh, npad=npad, cf=cf, dmax=dmax,
        CLS=CLS, XW=XW, SW=SW, F=F, B=B, G=G, NRANGE=NRANGE, rw=rw,
        x0_off=x0_off, x_off=x_off, x0_woff=x0_woff, x_woff=x_woff,
        expand_list=expand_list, m=m, widths=widths,
        Lmax=Lmax, LOmax=LOmax, s_off=s_off, so_off=so_off, s_woff=s_woff,
        mm_w=mm_w, mm_ovf=mm_ovf, G_w=G_w, g_base=g_base,
    )
    per_core = [emit_core_arrays(c, meta) for c in cores_raw]
    return meta, per_core


def emit_core_arrays(c, meta):
    cf, csh = meta["cf"], meta["csh"]
    CLS, F, G = meta["CLS"], meta["F"], meta["G"]
    x0_off, x_off = meta["x0_off"], meta["x_off"]
    x_woff, s_woff = meta["x_woff"], meta["s_woff"]
    s_off, so_off = meta["s_off"], meta["so_off"]
    rw = meta["rw"]

    def put(arr, prt, pos, tgt):
        arr[prt, pos] = tgt.astype(np.int16)

    # class rank of each source within (p, w, d)
    cls_key = (c["usrc_p"] * NW + c["win_of_usrc"]) * (int(c["d_src"].max()) + 1) + c["d_src"]
    cls_rank = _cumcount(cls_key)

    # ls1
    ls1 = np.full((P, cf), -1, np.int16)
    tgt = x0_off[c["win_of_usrc"], c["d_src"]] + cls_rank
    assert tgt.max() < CLS
    put(ls1, c["usrc_p"], c["usrc_q"], tgt)

    # X position per edge
    r_in_src = _cumcount(c["src_inv"])
    si = c["src_inv"]
    xpos = x_off[c["w_e"], c["d_src"][si]] + cls_rank[si] * c["d_src"][si] + r_in_src

    ls2 = []
    for w in range(NW):
        wlen = int(x_woff[w + 1] - x_woff[w])
        a2 = np.full((P, wlen), -1, np.int16)
        selm = (c["w_e"] == w) & c["main"]
        xl = xpos[selm] - x_woff[w]
        t2 = (c["b_e"][selm] - w * BLKW) * P + c["j_e"][selm]
        put(a2, c["p_s"][selm], xl, t2)
        om = c["w_e"][c["oi"]] == w          # mask over oi order
        xo = xpos[c["oi"]][om] - x_woff[w]
        to = BPW * P + c["t_o"][om]
        put(a2, c["p_s"][c["oi"]][om], xo, to)
        ls2.append(a2)

    # ls3 (main): input XT[:, w*BLKW*128 : +BPW*128], partition j
    ls3 = []
    r_v = c["c_v"] // rw
    for w in range(NW):
        wlen = BPW * P
        slen = int(s_woff[w + 1] - s_woff[w])
        arr = np.full((P, wlen), -1, np.int16)
        selm = (c["w_e"] == w) & c["main"]
        ipos = (c["b_e"][selm] - w * BLKW) * P + c["p_s"][selm]
        t3 = (
            s_off[w, r_v[selm], c["l_e"][selm]]
            + (c["c_v"][selm] - r_v[selm] * rw)
            - s_woff[w]
        )
        assert len(t3) == 0 or (t3.min() >= 0 and t3.max() < slen)
        put(arr, c["j_e"][selm], ipos, t3)
        ls3.append(arr)

    # lsa call w: input XT ovf block (w*BLKW+BPW) [P, 128] -> XO chunk G_w blocks
    oi = c["oi"]
    lsa = []
    for w in range(NW):
        arr = np.full((P, P), -1, np.int16)
        if len(oi):
            sel = c["w_e"][oi] == w
            ipos = c["p_s"][oi][sel]
            ta = c["g_o"][sel] * P + c["jo"][sel]
            put(arr, c["t_o"][sel], ipos, ta)
        lsa.append(arr)

    # lsb: XOT [P, G*128] -> SM ovf window at (j*, so_off + col)
    g_base = meta["g_base"]
    slen_o = int(s_woff[NW + 1] - s_woff[NW])
    lsb = np.full((P, G * P), -1, np.int16)
    if len(oi) and slen_o:
        gg = g_base[c["w_e"][oi]] + c["g_o"]
        ipos = gg * P + c["t_o"]
        tb = (
            so_off[r_v[oi], c["lo_"]]
            + (c["c_v"][oi] - r_v[oi] * rw)
            - s_woff[NW]
        )
        assert tb.min() >= 0 and tb.max() < slen_o
        put(lsb, c["jo"], ipos, tb)

    return dict(ls1=ls1, ls2=ls2, ls3=ls3, lsa=lsa, lsb=lsb)


# ──────────────────────────────────────────────────────────────────────
# numpy emulation (bf16 values, 1 int16 elem per value)
# ──────────────────────────────────────────────────────────────────────

def _emu_ls(data_i16, idx_i16, num_elems):
    Pp, n = idx_i16.shape
    assert data_i16.shape == (Pp, n)
    out = np.zeros((Pp, num_elems), np.int16)
    for p in range(Pp):
        ii = idx_i16[p].astype(np.int64)
        valid = ii >= 0
        assert len(np.unique(ii[valid])) == valid.sum(), "dup idx"
        out[p, ii[valid]] = data_i16[p, valid]
    return out


def emulate_round(w_full, meta, arrs):
    """w_full: [P, cf] float32 (will be cast bf16). Returns psum [SEGS, csh] f32."""
    cf, csh = meta["cf"], meta["csh"]
    CLS, XW, SW, F, B, G = (meta[k] for k in ("CLS", "XW", "SW", "F", "B", "G"))
    x_woff, s_woff = meta["x_woff"], meta["s_woff"]

    d16 = np.ascontiguousarray(w_full.astype(BF16)).view(np.int16)
    x0 = _emu_ls(d16, arrs["ls1"], CLS).view(BF16)

    x = np.zeros((P, XW), BF16)
    for (o0, mm, d, o, w) in meta["expand_list"]:
        x[:, o : o + mm * d] = np.repeat(x0[:, o0 : o0 + mm], d, axis=1)

    x2 = np.zeros((P, F), BF16)
    for w in range(NW):
        lo_, hi = int(x_woff[w]), int(x_woff[w + 1])
        seg = np.ascontiguousarray(x[:, lo_:hi]).view(np.int16)
        o = _emu_ls(seg, arrs["ls2"][w], BLKW * P).view(BF16)
        x2[:, w * BLKW * P : (w + 1) * BLKW * P] = o

    xt = np.zeros((P, F), BF16)
    for b in range(B):
        xt[:, b * P : (b + 1) * P] = x2[:, b * P : (b + 1) * P].T

    sm = np.zeros((P, SW), BF16)
    for w in range(NW):
        sl = int(s_woff[w + 1] - s_woff[w])
        seg = np.ascontiguousarray(xt[:, w * BLKW * P : w * BLKW * P + BPW * P]).view(np.int16)
        o = _emu_ls(seg, arrs["ls3"][w], sl).view(BF16)
        sm[:, int(s_woff[w]) : int(s_woff[w]) + sl] = o

    assert int(s_woff[NW + 1] - s_woff[NW]) == 0, "ovf path removed"

    psum = np.zeros((SEGS, csh), np.float32)
    smf = sm.astype(np.float32)
    for lst in (meta["mm_w"][0], meta["mm_w"][1], meta["mm_ovf"]):
        for (so, wd, po) in lst:
            psum[:, po : po + wd] += smf[:, so : so + wd].reshape(SEGS, R4, wd).sum(axis=1)
    return psum


# ─── preprocessing glue ───


def next_pow2(x):
    p = 1
    while p < x:
        p *= 2
    return p


def preprocess(x, edge_index, batch, nc_count=8, n_graphs=128):
    n_nodes = x.shape[0]
    row = np.asarray(edge_index[0], np.int64)
    col = np.asarray(edge_index[1], np.int64)
    batch = np.asarray(batch, np.int64)

    csh, sh, npad, cf = build_layout(n_nodes, nc_count)
    deg = np.bincount(col, minlength=npad).astype(np.int64)
    flat, layout = relabel(deg, n_nodes, nc_count)
    re, ve = flat[row], flat[col]

    cores_raw = []
    for c in range(nc_count):
        m = (ve // sh) == c
        cores_raw.append(build_core(c, re[m], ve[m], layout))
    meta, per_core = finalize_cores(cores_raw, layout)

    # device-order node arrays
    inv = np.empty(npad, np.int64)          # flat -> original id
    inv[flat] = np.arange(npad)
    deg_dev = deg[inv].astype(np.float64)   # deg at device flat position
    batch_dev = np.full(npad, -1, np.int64)
    batch_dev[flat[:n_nodes]] = batch[:n_nodes]

    dinv_dev = np.where(deg_dev > 0, deg_dev ** -0.5, 0.0).astype(np.float32)

    # ---- pooling structures ----
    g0 = np.zeros(nc_count, np.int64)
    ngl = np.zeros(nc_count, np.int64)
    wg_max = 0
    for c in range(nc_count):
        bd = batch_dev[c * sh:(c + 1) * sh]
        real = bd >= 0
        gmin, gmax = (int(bd[real].min()), int(bd[real].max())) if real.any() else (0, 0)
        g0[c], ngl[c] = gmin, gmax - gmin + 1
        fin = np.arange(sh)
        s = fin // csh
        cnt = np.zeros((SEGS, int(ngl[c])), np.int64)
        np.add.at(cnt, (s[real], bd[real] - gmin), 1)
        wg_max = max(wg_max, int(cnt.max()))
    NGLP = int(ngl.max())
    # pool over [128, CP8] layout: partition p = chunk*16 + s, chunk = c // CP8
    CP8 = cdiv(csh, 8)
    pool_idx = []
    gms = [np.zeros((P, P), np.float32) for _ in range(2)]
    w8_max = 0
    ranks = []
    for c in range(nc_count):
        bd = batch_dev[c * sh:(c + 1) * sh]
        fin = np.arange(sh)
        s, cc = fin // csh, fin % csh
        lg = bd - g0[c]
        pp_ = (cc // CP8) * SEGS + s
        pos = cc % CP8
        rank = np.zeros(sh, np.int64)
        real = bd >= 0
        key = pp_ * 4096 + lg
        rank[real] = _cumcount(key[real])
        ranks.append((pp_, pos, lg, rank, real))
        if real.any():
            w8_max = max(w8_max, int(rank[real].max()) + 1)
    W8 = next_pow2(w8_max)
    PH8 = NGLP * W8
    assert PH8 <= 2046, f"pool window {PH8}"
    assert NGLP <= 32
    for c in range(nc_count):
        pp_, pos, lg, rank, real = ranks[c]
        a = np.full((P, CP8), -1, np.int16)
        tgt = lg[real] * W8 + rank[real]
        a[pp_[real], pos[real]] = tgt.astype(np.int16)
        pool_idx.append(a)
        for li in range(NGLP):
            g = g0[c] + li
            if li < int(ngl[c]) and g < n_graphs:
                fp = c * 32 + li
                gms[fp % 2][fp // 2, g] = 1.0

    pool_meta = dict(NGLP=NGLP, W8=W8, PH8=PH8, CP8=CP8, g0=g0)
    return meta, per_core, pool_meta, pool_idx, gms, flat, dinv_dev, layout


def make_inputs(meta, pool_meta, per_core, pool_idx, gms, flat, dinv_dev,
                x, weights, n_max, n_graphs=128):
    """Build per-core in_maps. weights = dict(W1..Wl, b1..bl)."""
    csh, sh, npad, cf = meta["csh"], meta["sh"], meta["npad"], meta["cf"]
    nc_count = meta["nc"]
    n_nodes = x.shape[0]

    # x in device order, transposed: xT_dev[c] = [128, sh], bf16
    xdev = np.zeros((npad, x.shape[1]), np.float32)
    xdev[flat[:n_nodes]] = x
    dinvf = dinv_dev.reshape(P, cf)
    slo = int(meta["s_woff"][NW + 1] - meta["s_woff"][NW])

    wpack = np.zeros((64, 327), np.float32)
    wpack[:, 0:128] = weights["W1"].T
    wpack[:, 128:192] = weights["W2"].T
    wpack[:, 192:256] = weights["W3"].T
    wpack[:, 256:320] = weights["W4"].T
    wpack[:, 320:321] = weights["Wl"].reshape(64, 1)
    for k in range(1, 5):
        wpack[:, 320 + k:321 + k] = np.asarray(weights[f"b{k}"], np.float32).reshape(64, 1)
    wpack[0, 325] = float(np.asarray(weights["bl"]).ravel()[0])
    wpack[0, 326] = 1.0 / np.float32(n_max)
    bfpack = np.zeros((P, 144), np.float32)
    bfpack[:, 0:128] = np.eye(P)
    bfpack[:, 128:144] = np.repeat(np.eye(SEGS), R4, axis=0)
    gmpack = np.concatenate([gms[0], gms[1]], axis=1)

    in_maps = []
    for c in range(nc_count):
        im = dict(
            xT=np.ascontiguousarray(xdev[c * sh:(c + 1) * sh].T).astype(BF16),
            dinvf=dinvf.astype(np.float32),
            dpack=np.concatenate([
                dinv_dev[c * sh:(c + 1) * sh].reshape(SEGS, csh),
                (dinv_dev[c * sh:(c + 1) * sh] ** 2).reshape(SEGS, csh)],
                axis=1).astype(np.float32),
            ls1p=per_core[c]["ls1"],
            ls2p=np.concatenate(per_core[c]["ls2"], axis=1),
            ls3p=np.concatenate(per_core[c]["ls3"], axis=1),
            wpack=wpack,
            bfpack=bfpack.astype(BF16),
            gmpack=gmpack.astype(np.float32),
        )
        if slo > 0:
            im["lsb"] = per_core[c]["lsb"]
            for w in range(NW):
                im[f"lsa_{w}"] = per_core[c]["lsa"][w]
        im["pool8"] = pool_idx[c]
        in_maps.append(im)
    return in_maps


def reference_numpy(x, edge_index, batch, weights, n_graphs=128):
    """Direct numpy reference of the original model."""
    row = np.asarray(edge_index[0]); col = np.asarray(edge_index[1])
    N = x.shape[0]
    deg = np.bincount(col, minlength=N).astype(np.float64)
    dinv = np.where(deg > 0, deg ** -0.5, 0.0)
    norm = dinv[row] * dinv[col]
    h = x.astype(np.float64)
    for k in range(1, 5):
        W = weights[f"W{k}"]
        b = weights[f"b{k}"]
        hw = h @ W
        msg = norm[:, None] * hw[row]
        out = np.zeros((N, hw.shape[1]))
        np.add.at(out, col, msg)
        h = out + b
    sums = np.zeros((n_graphs, h.shape[1]))
    np.add.at(sums, batch, h)
    counts = np.bincount(batch, minlength=n_graphs)
    pooled = sums / counts.max()
    return (pooled @ weights["Wl"] + weights["bl"]).astype(np.float32)


# ─── device kernel ───
from contextlib import ExitStack

import concourse.bass as bass
import concourse.tile as tile
from concourse import bacc, mybir

FP32 = mybir.dt.float32
BF = mybir.dt.bfloat16
I16 = mybir.dt.int16
AT = mybir.ActivationFunctionType
OP = mybir.AluOpType


def build_kernel(meta, pool_meta, n_graphs=128):
    csh, sh, npad, cf = meta["csh"], meta["sh"], meta["npad"], meta["cf"]
    CLS, XW, SW, F, B, G = (meta[k] for k in ("CLS", "XW", "SW", "F", "B", "G"))
    x0_woff, x_woff, s_woff = meta["x0_woff"], meta["x_woff"], meta["s_woff"]
    G_w, g_base = meta["G_w"], meta["g_base"]
    mm_w, mm_ovf = meta["mm_w"], meta["mm_ovf"]
    NGLP, W8 = pool_meta["NGLP"], pool_meta["W8"]
    PH8, CP8 = pool_meta["PH8"], pool_meta["CP8"]
    core_ids = list(range(meta["nc"]))

    nc = bacc.Bacc("TRN2", target_bir_lowering=False, debug=False,
                   num_devices=meta["nc"])

    def din(name, shape, dt=FP32):
        return nc.declare_dram_parameter(name, list(shape), dt, isOutput=False)

    slo_pre = int(s_woff[NW + 1] - s_woff[NW])
    xw_tot = int(x_woff[NW])
    # ---- inputs ----
    xT_in = din("xT", [P, sh], BF)
    dinvf_in = din("dinvf", [P, cf])
    dpack_in = din("dpack", [SEGS, 2 * csh])
    ls1p_in = din("ls1p", [P, cf], I16)
    ls2p_in = din("ls2p", [P, xw_tot], I16)
    ls3p_in = din("ls3p", [P, NW * BPW * P], I16)
    if slo_pre > 0:
        lsa_in = [din(f"lsa_{w}", [P, P], I16) for w in range(NW)]
        lsb_in = din("lsb", [P, G * P], I16)
    pool8_in = din("pool8", [P, CP8], I16)
    wpack_in = din("wpack", [64, 327])
    bfpack_in = din("bfpack", [P, 144], BF)
    gmpack_in = din("gmpack", [P, 256])
    out_ext = nc.declare_dram_parameter("out", [n_graphs], FP32, isOutput=True)

    # ---- internal DRAM ----
    sh_dram = nc.dram_tensor("sh_dram", [sh], BF)
    full_dram = nc.dram_tensor("full_dram", [npad], BF, addr_space="Shared")
    part_dram = nc.dram_tensor("part_dram", [32], FP32)
    warm_in = nc.dram_tensor("warm_in", [32], FP32)
    warm_out = nc.dram_tensor("warm_out", [256], FP32, addr_space="Shared")
    partall_dram = nc.dram_tensor("partall_dram", [256], FP32, addr_space="Shared")

    slo = int(s_woff[NW + 1] - s_woff[NW])

    with tile.TileContext(nc) as tc:
        with ExitStack() as ctx:
            pool = ctx.enter_context(tc.tile_pool(name="p", bufs=1))
            tp = ctx.enter_context(tc.tile_pool(name="tp", bufs=3, space="PSUM"))
            up = ctx.enter_context(tc.tile_pool(name="up", bufs=1, space="PSUM"))

            # persistent tiles
            state = pool.tile([P, cf], BF)
            wbuf = pool.tile([P, cf], BF)
            tbuf = pool.tile([P, cf], FP32)
            dinvf = pool.tile([P, cf], FP32)
            dpk = pool.tile([SEGS, 2 * csh], FP32)
            dinvs = dpk[:, 0:csh]
            dinv2s = dpk[:, csh:2 * csh]
            bdfs = pool.tile([SEGS, csh], FP32)
            x0 = pool.tile([P, CLS], BF)
            xbuf = pool.tile([P, XW], BF)
            x2 = [pool.tile([P, BLKW * P], BF, name=f"x2_{w}") for w in range(NW)]
            xt = [pool.tile([P, BLKW * P], BF, name=f"xt_{w}") for w in range(NW)]
            sm = pool.tile([P, SW], BF)
            xo = pool.tile([P, G * P], BF)
            xot = pool.tile([P, G * P], BF)
            u_bf = pool.tile([SEGS, csh], BF)
            s4_bf = pool.tile([SEGS, CP8 * 8], BF)
            s4r = pool.tile([P, CP8], BF)
            t4 = pool.tile([SEGS, csh], FP32)
            bfp = pool.tile([P, 144], BF)
            ident = bfp[:, 0:128]
            sel = bfp[:, 128:144]
            ones16 = pool.tile([SEGS, 1], FP32)
            ones128 = pool.tile([1, P], FP32)
            gmp = pool.tile([P, 256], FP32)
            gm = [gmp[:, 0:128], gmp[:, 128:256]]
            ls1p = pool.tile([P, cf], I16)
            ls2p = pool.tile([P, xw_tot], I16)
            ls2 = [ls2p[:, int(x_woff[w]):int(x_woff[w + 1])] for w in range(NW)]
            ls3p = pool.tile([P, NW * BPW * P], I16)
            ls3 = [ls3p[:, w * BPW * P:(w + 1) * BPW * P] for w in range(NW)]
            if slo_pre > 0:
                lsa = [pool.tile([P, P], I16, name=f"lsat{w}") for w in range(NW)]
                lsb = pool.tile([P, G * P], I16)
            plidx8 = pool.tile([P, CP8], I16)
            poolb8 = pool.tile([P, PH8], BF)
            poolf8 = pool.tile([P, PH8], FP32)
            ones128c = pool.tile([P, 1], FP32)
            part_sb = pool.tile([1, 32], FP32)
            partall = pool.tile([P, 2], FP32)
            outrow = pool.tile([1, n_graphs], FP32)
            stage = pool.tile([1, sh], BF)
            wpk = pool.tile([64, 327], FP32)
            wts = {
                "w1t": wpk[:, 0:128],
                "w2t": wpk[:, 128:192],
                "w3t": wpk[:, 192:256],
                "w4t": wpk[:, 256:320],
                "wl": wpk[:, 320:321],
            }
            bs = [wpk[:, 321 + k:322 + k] for k in range(4)]
            blt = wpk[0:1, 325:326]
            invn = wpk[0:1, 326:327]
            cvec = {
                "c3": pool.tile([64, 1], FP32, name="c3t"),
                "c2": pool.tile([64, 1], FP32, name="c2t"),
                "c1": pool.tile([64, 1], FP32, name="c1t"),
                "c0": pool.tile([128, 1], FP32, name="c0t"),
            }
            c0b = pool.tile([128, 1], BF)
            betas = pool.tile([1, 4], FP32)
            betas16 = pool.tile([SEGS, 4], FP32)

            # ---- warmup collective + early weight load ----
            warmsb = pool.tile([1, 32], FP32)
            nc.vector.memset(warmsb[:], 0.0)
            nc.vector.memset(ones16[:], 1.0)
            nc.vector.memset(ones128[:], 1.0)
            nc.vector.memset(ones128c[:], 1.0)
            if CP8 * 8 > csh:
                nc.vector.memset(s4_bf[:, csh:CP8 * 8], 0.0)
            nc.sync.dma_start(warm_in[:].rearrange("(a b) -> a b", a=1),
                              warmsb[:])
            nc.sync.dma_start(wpk[:], wpack_in[:])
            nc.gpsimd.collective_compute(
                "AllGather", OP.bypass, replica_groups=[core_ids],
                ins=[warm_in[:]], outs=[warm_out[:]],
            )

            # ---- c chain + betas ----
            pc = tp.tile([128, 8], FP32, tag="ptr")
            nc.tensor.matmul(pc[0:64, 0:1], wts["w4t"][:], wts["wl"][:],
                             start=True, stop=True)
            nc.vector.tensor_copy(cvec["c3"][:], pc[0:64, 0:1])
            nc.tensor.matmul(pc[0:64, 1:2], wts["w3t"][:], cvec["c3"][:],
                             start=True, stop=True)
            nc.vector.tensor_copy(cvec["c2"][:], pc[0:64, 1:2])
            nc.tensor.matmul(pc[0:64, 2:3], wts["w2t"][:], cvec["c2"][:],
                             start=True, stop=True)
            nc.vector.tensor_copy(cvec["c1"][:], pc[0:64, 2:3])
            nc.tensor.matmul(pc[0:128, 3:4], wts["w1t"][:], cvec["c1"][:],
                             start=True, stop=True)
            nc.vector.tensor_copy(cvec["c0"][:], pc[0:128, 3:4])
            nc.vector.tensor_copy(c0b[:], cvec["c0"][:])
            pb = tp.tile([1, 4], FP32, tag="ptr")
            for k, cn in enumerate(["c1", "c2", "c3"]):
                nc.tensor.matmul(pb[0:1, k:k + 1], bs[k][:], cvec[cn][:],
                                 start=True, stop=True)
            nc.tensor.matmul(pb[0:1, 3:4], bs[3][:], wts["wl"][:],
                             start=True, stop=True)
            nc.vector.tensor_copy(betas[:], pb[:])
            pbb = tp.tile([P, 4], FP32, tag="ptr")
            nc.tensor.matmul(pbb[:], ones128[:], betas[:], start=True, stop=True)
            nc.vector.tensor_copy(betas16[:], pbb[0:SEGS, :])

            # ---- s0 = x @ c0 (bf16, 8 chunks, pipelined) ----
            NCH = 16
            chw = sh // NCH
            assert chw % 2 == 0
            pw2 = chw // 2
            qeng = [nc.sync, nc.scalar, nc.gpsimd]
            xfull = pool.tile([P, sh], BF)
            for q in range(NCH):
                qeng[q % 3].dma_start(xfull[:, q * chw:(q + 1) * chw],
                                      xT_in[:, q * chw:(q + 1) * chw])
            shv = sh_dram[:].rearrange("(a b) -> a b", a=1)
            for q in range(NCH):
                for pi in range(2):
                    ps0 = tp.tile([P, 512], FP32, tag="ptr", name=f"ps0_{q}_{pi}")
                    nc.tensor.matmul(
                        ps0[0:1, 0:pw2], c0b[:],
                        xfull[:, q * chw + pi * pw2: q * chw + (pi + 1) * pw2],
                        start=True, stop=True)
                    eng = nc.vector if (q + pi) % 2 == 0 else nc.scalar
                    if eng is nc.vector:
                        nc.vector.tensor_copy(
                            stage[:, q * chw + pi * pw2: q * chw + (pi + 1) * pw2],
                            ps0[0:1, 0:pw2])
                    else:
                        nc.scalar.activation(
                            stage[:, q * chw + pi * pw2: q * chw + (pi + 1) * pw2],
                            ps0[0:1, 0:pw2], AT.Copy)
                qeng[q % 3].dma_start(
                    shv[:, q * chw:(q + 1) * chw], stage[:, q * chw:(q + 1) * chw])

            # ---- index-table loads, issue spread across engine queues ----
            nc.scalar.dma_start(ls1p[:], ls1p_in[:])
            nc.gpsimd.dma_start(ls2p[:], ls2p_in[:])
            nc.sync.dma_start(ls3p[:], ls3p_in[:])
            nc.scalar.dma_start(dinvf[:], dinvf_in[:])
            nc.gpsimd.dma_start(dpk[:], dpack_in[:])
            nc.sync.dma_start(bfp[:], bfpack_in[:])
            nc.scalar.dma_start(gmp[:], gmpack_in[:])
            if slo_pre > 0:
                for w in range(NW):
                    nc.sync.dma_start(lsa[w][:], lsa_in[w][:])
                nc.scalar.dma_start(lsb[:], lsb_in[:])
            nc.gpsimd.dma_start(plidx8[:], pool8_in[:])

            # ---- state rounds ----
            def allgather_state(dst):
                nc.gpsimd.collective_compute(
                    "AllGather", OP.bypass, replica_groups=[core_ids],
                    ins=[sh_dram[:]], outs=[full_dram[:]],
                )
                nc.sync.dma_start(
                    dst[:], full_dram[:].rearrange("(p c) -> p c", p=P))

            allgather_state(state)

            # PSUM accumulation is bank-scoped: the first matmul into a bank
            # (start=True) clears the bank's has_written bits; later matmuls
            # (start=False) overwrite-on-first-touch / accumulate-where-set
            # per element. So per round emit exactly one start and one stop
            # per bank, regardless of region interleaving.
            bank_total = {}
            for lst in (mm_w + [mm_ovf]):
                for (so, wd, po) in lst:
                    b = po // 512
                    assert (po + wd - 1) // 512 == b
                    bank_total[b] = bank_total.get(b, 0) + 1

            for rnd in range(4):
                if rnd == 0:
                    # w = bf16(state * dinv) in fp32 (receiver side, rnd 0 only)
                    nc.vector.tensor_copy(tbuf[:], state[:])
                    nc.vector.tensor_tensor(wbuf[:], tbuf[:], dinvf[:], OP.mult)
                # LS1
                nc.gpsimd.local_scatter(
                    x0[:].bitcast(I16), wbuf[:].bitcast(I16), ls1p[:],
                    channels=P, num_elems=CLS, num_idxs=cf)
                if rnd < 3:
                    # bdfs for this round's output state (off critical path)
                    nc.vector.tensor_scalar(
                        bdfs[:], dinvs[:], betas16[:, rnd:rnd + 1], None, OP.mult)
                # expand + LS2 per window
                for w in range(NW):
                    ei = 0
                    for (o0, mm_, d, o, we) in meta["expand_list"]:
                        if we != w:
                            continue
                        src = x0[:, o0:o0 + mm_].unsqueeze(2).broadcast_to([P, mm_, d])
                        dst = xbuf[:, o:o + mm_ * d].rearrange("p (m d) -> p m d", d=d)
                        if ei % 2 == 0:
                            nc.vector.tensor_copy(dst, src)
                        else:
                            nc.scalar.activation(dst, src, AT.Copy)
                        ei += 1
                    lo_, hi = int(x_woff[w]), int(x_woff[w + 1])
                    nc.gpsimd.local_scatter(
                        x2[w][:].bitcast(I16),
                        xbuf[:, lo_:hi].bitcast(I16), ls2[w][:],
                        channels=P, num_elems=BLKW * P,
                        num_idxs=(hi - lo_))
                    # transposes of this window's blocks (PE, overlaps next LS2)
                    for b0 in range(0, BLKW, 4):
                        nb = min(4, BLKW - b0)
                        pt = tp.tile([P, 512], BF, tag="ptr", name=f"pt{rnd}_{w}_{b0}")
                        for k in range(nb):
                            b = b0 + k
                            nc.tensor.transpose(pt[:, k * P:(k + 1) * P],
                                                x2[w][:, b * P:(b + 1) * P], ident[:])
                        if (b0 // 4) % 2 == 0:
                            nc.vector.tensor_copy(xt[w][:, b0 * P:(b0 + nb) * P],
                                                  pt[:, 0:nb * P])
                        else:
                            nc.scalar.activation(xt[w][:, b0 * P:(b0 + nb) * P],
                                                 pt[:, 0:nb * P], AT.Copy)
                # LS3 + LSA per window; all sel matmuls accumulate into ONE pu
                pu = up.tile([SEGS, csh], FP32, tag="pu", name=f"pu{rnd}")
                bank_seen = {}
                def emit_mms(lst):
                    for (so, wd, po) in lst:
                        b = po // 512
                        seen = bank_seen.get(b, 0)
                        bank_seen[b] = seen + 1
                        nc.tensor.matmul(
                            pu[:, po:po + wd], sel[:], sm[:, so:so + wd],
                            start=(seen == 0),
                            stop=(seen + 1 == bank_total[b]),
                            skip_group_check=True)
                for w in range(NW):
                    sl = int(s_woff[w + 1] - s_woff[w])
                    nc.gpsimd.local_scatter(
                        sm[:, int(s_woff[w]):int(s_woff[w]) + sl].bitcast(I16),
                        xt[w][:].bitcast(I16),
                        ls3[w][:], channels=P, num_elems=sl,
                        num_idxs=BPW * P)
                    if slo > 0:
                        gw = int(G_w[w])
                        gb = int(g_base[w])
                        nc.gpsimd.local_scatter(
                            xo[:, gb * P:(gb + gw) * P].bitcast(I16),
                            xt[w][:, BPW * P:(BPW + 1) * P].bitcast(I16),
                            lsa[w][:], channels=P, num_elems=gw * P,
                            num_idxs=P)
                        # XO transposes for this window's g-blocks (PE)
                        assert gw <= 4
                        pt = tp.tile([P, 512], BF, tag="ptr", name=f"po{rnd}_{w}")
                        for k in range(gw):
                            g = gb + k
                            nc.tensor.transpose(pt[:, k * P:(k + 1) * P],
                                                xo[:, g * P:(g + 1) * P], ident[:])
                        nc.scalar.activation(xot[:, gb * P:(gb + gw) * P],
                                             pt[:, 0:gw * P], AT.Copy)
                    if w == NW - 1:
                        emit_mms([e for e in mm_w[w] if e[2] < 512]
                                 + [e for e in mm_w[w] if e[2] >= 512])
                    else:
                        emit_mms(mm_w[w])
                # ovf: LSB + ovf matmuls
                if slo > 0:
                    nc.gpsimd.local_scatter(
                        sm[:, int(s_woff[NW]):int(s_woff[NW]) + slo].bitcast(I16),
                        xot[:].bitcast(I16), lsb[:],
                        channels=P, num_elems=slo, num_idxs=G * P)
                    emit_mms(mm_ovf)
                assert bank_seen == bank_total

                # sender-side state math: w_next = bf16(u*dinv^2 + beta*dinv)
                # split per psum bank so bank A's math/send overlaps bank B's
                # matmuls.
                if rnd < 3:
                    shv2 = sh_dram[:].rearrange("(s c) -> s c", s=SEGS)
                    for (a, b2) in ((0, 512), (512, csh)):
                        nc.vector.tensor_tensor(
                            t4[:, a:b2], pu[:, a:b2], dinv2s[:, a:b2], OP.mult)
                        nc.vector.tensor_tensor(
                            u_bf[:, a:b2], t4[:, a:b2], bdfs[:, a:b2], OP.add)
                        nc.gpsimd.dma_start(shv2[:, a:b2], u_bf[:, a:b2])
                    allgather_state(wbuf)
                else:
                    nc.vector.tensor_tensor(t4[:], pu[:], dinvs[:], OP.mult)
                    nc.vector.tensor_scalar(
                        s4_bf[:, 0:csh], t4[:], betas16[:, 3:4], None, OP.add)

            # ---- pooling (on [128, CP8] reshaped shard) ----
            for k in range(8):
                qeng[k % 3].dma_start(s4r[k * SEGS:(k + 1) * SEGS, :],
                                      s4_bf[:, k * CP8:(k + 1) * CP8])
            nc.gpsimd.local_scatter(
                poolb8[:].bitcast(I16), s4r[:].bitcast(I16), plidx8[:],
                channels=P, num_elems=PH8, num_idxs=CP8)
            nc.vector.tensor_copy(poolf8[:], poolb8[:])
            wgp = W8
            a = poolf8[:].rearrange("p (g t) -> p g t", t=W8)
            while wgp > 1:
                hw = wgp // 2
                nc.vector.tensor_tensor(
                    a[:, :, 0:hw], a[:, :, 0:hw], a[:, :, hw:wgp], OP.add)
                wgp = hw
            pp = tp.tile([1, 512], FP32, tag="ptr")
            nc.tensor.matmul(pp[0:1, 0:NGLP], ones128c[:], a[:, :, 0],
                             start=True, stop=True)
            nc.vector.memset(part_sb[:], 0.0)
            nc.vector.tensor_copy(part_sb[:, 0:NGLP], pp[0:1, 0:NGLP])
            nc.gpsimd.dma_start(part_dram[:].rearrange("(a b) -> a b", a=1),
                              part_sb[:])
            nc.gpsimd.collective_compute(
                "AllGather", OP.bypass, replica_groups=[core_ids],
                ins=[part_dram[:]], outs=[partall_dram[:]],
            )
            nc.sync.dma_start(partall[:],
                              partall_dram[:].rearrange("(p c) -> p c", p=P))
            po_ = tp.tile([1, n_graphs], FP32, tag="ptr")
            nc.tensor.matmul(po_[:], partall[:, 0:1], gm[0][:],
                             start=True, stop=False)
            nc.tensor.matmul(po_[:], partall[:, 1:2], gm[1][:],
                             start=False, stop=True)
            nc.vector.tensor_copy(outrow[:], po_[:])
            nc.vector.tensor_scalar(outrow[:], outrow[:], invn[0:1, 0:1],
                                    None, OP.mult)
            nc.vector.tensor_scalar(outrow[:], outrow[:], blt[0:1, 0:1],
                                    None, OP.add)
            nc.sync.dma_start(out_ext[:].rearrange("(a b) -> a b", a=1),
                              outrow[:])
    return nc


# ─── entry point ───

def kernel(x, edge_index, batch, W1, b1, W2, b2, W3, b3, W4, b4, Wl, bl):
    from concourse.bass_utils import run_bass_kernel_spmd

    x = np.asarray(x, np.float32)
    edge_index = np.asarray(edge_index)
    batch = np.asarray(batch)
    weights = dict(W1=np.asarray(W1, np.float32), W2=np.asarray(W2, np.float32),
                   W3=np.asarray(W3, np.float32), W4=np.asarray(W4, np.float32),
                   Wl=np.asarray(Wl, np.float32),
                   b1=np.asarray(b1, np.float32), b2=np.asarray(b2, np.float32),
                   b3=np.asarray(b3, np.float32), b4=np.asarray(b4, np.float32),
                   bl=np.asarray(bl, np.float32))
    n_graphs = 128

    meta, per_core, pool_meta, pool_idx, gms, flat, dinv_dev, layout = \
        preprocess(x, edge_index, batch, 8, n_graphs)
    n_max = int(np.bincount(np.asarray(batch, np.int64),
                            minlength=n_graphs).max())
    in_maps = make_inputs(meta, pool_meta, per_core, pool_idx, gms, flat,
                          dinv_dev, x, weights, n_max, n_graphs)
    nc = build_kernel(meta, pool_meta, n_graphs)
    nc.finalize()
    res = run_bass_kernel_spmd(nc, in_maps, core_ids=list(range(8)),
                               trace=False)
    return res.results[0]["out"].reshape(n_graphs, 1).astype(np.float32)
